# revision 20
# baseline (speedup 1.0000x reference)
"""Trainium2 Bass kernel for nn_Branch_4887672782920 (PCT-style point cloud net).

Data-parallel over batch B=32 across 8 NeuronCores (4 clouds/core), two SPMD
launches with in-kernel cross-core AllReduce for the training-mode BatchNorm
statistics.  The serial farthest-point-sampling / kNN index selection (pure
fp32 index math) runs on host between the launches; all feature gathers,
matmuls, batchnorms, attention and pooling run on device.
"""
import numpy as np

import concourse.bass as bass
import concourse.bacc as bacc
import concourse.mybir as mybir
from concourse.bass_utils import run_bass_kernel_spmd
from concourse.masks import make_identity
from concourse.tile import TileContext
from concourse.vector_clock import ScopedClock

F32 = mybir.dt.float32
I32 = mybir.dt.int32
AX = mybir.AxisListType
OP = mybir.AluOpType
ACT = mybir.ActivationFunctionType

N_CORES = 8
B = 32
BL = B // N_CORES
N = 2048
S1, K1 = 512, 32
S2, K2 = 256, 32
EPS = 1e-5
RG = [list(range(N_CORES))]


class TC(TileContext):
    """Tail drain carries >2 sem waits; this walrus rejects that. Split it."""

    def _drain_and_barrier(self, tick_clock, wait_clock):
        import bass_rust

        drain_inst = self.nc.sync.drain()
        wait_clock.add_sem_waits(
            drain_inst.ins, ScopedClock({None: tick_clock.global_clock})
        )
        si = drain_inst.ins.sync_info
        waits = list(si.on_wait or [])
        if len(waits) > 2:
            si.on_wait = waits[:2]
            for i in range(2, len(waits), 2):
                d2 = self.nc.sync.drain()
                d2.ins.sync_info = bass_rust.SyncInfo(
                    on_wait=waits[i : i + 2], on_update=[]
                )
        self.nc.all_engine_barrier()
        assert self.sems is not None
        popped = self.nc._tile_sem_poison_stack.pop()
        assert popped is self._sem_poison
        self.nc.clear_and_free_semaphores(list(self.sems.allocated().values()))
        self.nc.all_engine_barrier()


def bc(ap, count, axis):
    """Stride-0 broadcast dim inserted at free position `axis` of an AP."""
    pairs = list(ap.ap)
    pairs.insert(axis, [0, count])
    return bass.AP(ap.tensor, ap.offset, pairs)


_CACHE = {}

_TILE_UID = [0]


def _patch_pools(*pools):
    for pool in pools:
        orig = pool.tile

        def tile(shape, dtype, _orig=orig, tag="", name=None, **kw):
            if name is None:
                if tag:
                    name = tag
                else:
                    _TILE_UID[0] += 1
                    name = f"u{_TILE_UID[0]}"
            return _orig(shape, dtype, tag=tag, name=name, **kw)

        pool.tile = tile


def _bn_coeffs(nc, sb, dram, stats, gb, count, tag, G=1):
    """AllReduce packed [128, 2G] (sum, sumsq per group) -> (s, t) [128, G]."""
    P = stats.shape[0]
    ib = dram.tile([P, 2 * G], F32, tag=f"{tag}_ib")
    ob = dram.tile([P, 2 * G], F32, tag=f"{tag}_ob")
    nc.sync.dma_start(ib[:], stats[:])
    nc.gpsimd.collective_compute(
        "AllReduce", OP.add, replica_groups=RG, ins=[ib[:].opt()], outs=[ob[:].opt()]
    )
    red = sb.tile([P, 2 * G], F32, tag=f"{tag}_red")
    nc.sync.dma_start(red[:], ob[:])
    s = sb.tile([P, G], F32, tag=f"{tag}_s")
    t = sb.tile([P, G], F32, tag=f"{tag}_t")
    mu = sb.tile([P, 1], F32, tag="bnc_mu")
    ex2 = sb.tile([P, 1], F32, tag="bnc_ex2")
    nvar = sb.tile([P, 1], F32, tag="bnc_nvar")
    sd = sb.tile([P, 1], F32, tag="bnc_sd")
    rstd = sb.tile([P, 1], F32, tag="bnc_rstd")
    tmp = sb.tile([P, 1], F32, tag="bnc_tmp")
    eps_t = sb.tile([P, 1], F32, tag="bnc_eps")
    nc.vector.memset(eps_t[:], EPS)
    for g in range(G):
        nc.vector.tensor_scalar_mul(mu[:], red[:, 2 * g : 2 * g + 1], 1.0 / count)
        nc.vector.tensor_scalar_mul(ex2[:], red[:, 2 * g + 1 : 2 * g + 2], 1.0 / count)
        nc.vector.scalar_tensor_tensor(
            out=nvar[:], in0=mu[:], scalar=mu[:], in1=ex2[:],
            op0=OP.mult, op1=OP.subtract)          # mu^2 - E[x^2]  (= -var)
        nc.scalar.activation(sd[:], nvar[:], ACT.Sqrt, bias=eps_t[:], scale=-1.0)
        nc.vector.reciprocal(rstd[:], sd[:])
        nc.vector.tensor_tensor(out=s[:, g : g + 1], in0=rstd[:],
                                in1=gb[:, 2 * g : 2 * g + 1], op=OP.mult)
        nc.vector.tensor_tensor(out=tmp[:], in0=mu[:], in1=s[:, g : g + 1], op=OP.mult)
        nc.vector.tensor_tensor(out=t[:, g : g + 1],
                                in0=gb[:, 2 * g + 1 : 2 * g + 2], in1=tmp[:],
                                op=OP.subtract)    # t = b - mu*s
    return s, t


# ---------------------------------------------------------------------------
# Launch 1: input MLP.  x7 [7, BL*2048] -> ptsT [BL, 64, 2048]
# ---------------------------------------------------------------------------
def build_l1():
    if "l1" in _CACHE:
        return _CACHE["l1"]
    nc = bacc.Bacc("TRN2", num_devices=N_CORES)
    x7 = nc.dram_tensor("x7", [7, BL * N], F32, kind="ExternalInput")
    w1T = nc.dram_tensor("w1T", [7, 64], F32, kind="ExternalInput")
    w2T = nc.dram_tensor("w2T", [64, 64], F32, kind="ExternalInput")
    gb1 = nc.dram_tensor("gb1", [64, 2], F32, kind="ExternalInput")
    gb2 = nc.dram_tensor("gb2", [64, 2], F32, kind="ExternalInput")
    ptsT_out = nc.dram_tensor("ptsT", [BL, 64, N], F32, kind="ExternalOutput")

    NCH = N // 512
    with TC(nc) as tc:
        with (
            tc.tile_pool(name="sb", bufs=1) as sb,
            tc.tile_pool(name="ps", bufs=4, space="PSUM") as ps,
            tc.tile_pool(name="dram", bufs=1, space="DRAM") as dram,
        ):
            _patch_pools(sb, ps, dram)
            xt = sb.tile([7, BL * N], F32)
            nc.sync.dma_start(xt[:], x7[:])
            w1t = sb.tile([7, 64], F32)
            nc.sync.dma_start(w1t[:], w1T[:])
            w2t = sb.tile([64, 64], F32)
            nc.sync.dma_start(w2t[:], w2T[:])
            gb1t = sb.tile([64, 2], F32)
            nc.sync.dma_start(gb1t[:], gb1[:])
            gb2t = sb.tile([64, 2], F32)
            nc.sync.dma_start(gb2t[:], gb2[:])
            sqs = sb.tile([64, 512], F32)
            sc = sb.tile([64, 1], F32)

            def layer(src, wt, gbt, tag):
                h = sb.tile([64, BL * N], F32, tag=f"{tag}_h")
                st = sb.tile([64, 2], F32, tag=f"{tag}_st")
                nc.vector.memset(st[:], 0.0)
                for b in range(BL):
                    for ch in range(NCH):
                        pt = ps.tile([64, 512], F32, tag="mm")
                        sl = slice((b * NCH + ch) * 512, (b * NCH + ch + 1) * 512)
                        nc.tensor.matmul(pt[:], wt[:], src[:, sl], start=True, stop=True)
                        nc.scalar.activation(h[:, sl], pt[:], ACT.Copy,
                                             accum_out=sc[:])
                        nc.vector.tensor_tensor(out=st[:, 0:1], in0=st[:, 0:1],
                                                in1=sc[:], op=OP.add)
                        nc.scalar.activation(sqs[:], pt[:], ACT.Square,
                                             accum_out=sc[:])
                        nc.vector.tensor_tensor(out=st[:, 1:2], in0=st[:, 1:2],
                                                in1=sc[:], op=OP.add)
                s, t = _bn_coeffs(nc, sb, dram, st, gbt[:], B * N, tag)
                hn = sb.tile([64, BL * N], F32, tag=f"{tag}_hn")
                for b in range(BL):
                    sl = slice(b * N, (b + 1) * N)
                    nc.scalar.activation(hn[:, sl], h[:, sl], ACT.Relu,
                                         bias=t[:, 0:1], scale=s[:, 0:1])
                return hn

            h1 = layer(xt, w1t, gb1t, "bn1")
            h2 = layer(h1, w2t, gb2t, "bn2")
            for b in range(BL):
                nc.sync.dma_start(ptsT_out[b], h2[:, b * N : (b + 1) * N])
    nc.compile()
    _CACHE["l1"] = nc
    return nc


# ---------------------------------------------------------------------------
# Host index math (FPS + kNN), replicating reference fp32 semantics.
# ---------------------------------------------------------------------------
def fps_np(xyz, npoint):
    Bn, Nn, _ = xyz.shape
    dist = np.full((Bn, Nn), 1e10, np.float32)
    far = np.zeros(Bn, np.int64)
    out = np.zeros((Bn, npoint), np.int64)
    ar = np.arange(Bn)
    for i in range(npoint):
        out[:, i] = far
        c = xyz[ar, far]
        d = ((xyz - c[:, None, :]) ** 2).sum(-1, dtype=np.float32)
        dist = np.minimum(dist, d)
        far = np.argmax(dist, axis=1)
    return out


def knn_np(new_xyz, xyz, k):
    d = (
        (new_xyz**2).sum(-1)[:, :, None]
        + (xyz**2).sum(-1)[:, None, :]
        - 2.0 * np.einsum("bsc,bnc->bsn", new_xyz, xyz)
    ).astype(np.float32)
    return np.argsort(d, axis=-1, kind="stable")[:, :, :k]


# ---------------------------------------------------------------------------
# Launch 2
# ---------------------------------------------------------------------------
def build_l2():
    if "l2" in _CACHE:
        return _CACHE["l2"]
    nc = bacc.Bacc("TRN2", num_devices=N_CORES)

    pts_rows = nc.dram_tensor("pts_rows", [BL * N, 64], F32, kind="ExternalInput")
    ix_g0 = nc.dram_tensor("ix_g0", [BL, 128, 4 * K1], I32, kind="ExternalInput")
    ix_c0 = nc.dram_tensor("ix_c0", [BL, 128, 4], I32, kind="ExternalInput")
    ix_g1 = nc.dram_tensor("ix_g1", [BL, 128, 2 * K2], I32, kind="ExternalInput")
    ix_c1 = nc.dram_tensor("ix_c1", [BL, 128, 2], I32, kind="ExternalInput")

    w01 = nc.dram_tensor("w01", [64, 2, 128], F32, kind="ExternalInput")
    w02 = nc.dram_tensor("w02", [128, 128], F32, kind="ExternalInput")
    gb0 = nc.dram_tensor("gb0", [128, 4], F32, kind="ExternalInput")
    w11 = nc.dram_tensor("w11", [2, 128, 256], F32, kind="ExternalInput")
    w12 = nc.dram_tensor("w12", [2, 128, 256], F32, kind="ExternalInput")
    gb1_ = nc.dram_tensor("gb1_", [128, 8], F32, kind="ExternalInput")
    pw1 = nc.dram_tensor("pw1", [2, 128, 256], F32, kind="ExternalInput")
    pw2 = nc.dram_tensor("pw2", [2, 128, 256], F32, kind="ExternalInput")
    gbpw = nc.dram_tensor("gbpw", [128, 8], F32, kind="ExternalInput")
    sa_wqk = nc.dram_tensor("sa_wqk", [4, 2, 128, 64], F32, kind="ExternalInput")
    sa_wv = nc.dram_tensor("sa_wv", [4, 2, 128, 256], F32, kind="ExternalInput")
    sa_wt = nc.dram_tensor("sa_wt", [4, 2, 128, 256], F32, kind="ExternalInput")
    sa_btp = nc.dram_tensor("sa_btp", [4, 2, 128, 1], F32, kind="ExternalInput")
    sa_gb = nc.dram_tensor("sa_gb", [4, 128, 4], F32, kind="ExternalInput")
    fw = nc.dram_tensor("fw", [10, 8, 128, 128], F32, kind="ExternalInput")
    gbf = nc.dram_tensor("gbf", [128, 16], F32, kind="ExternalInput")

    out = nc.dram_tensor("out", [BL, 8, 128], F32, kind="ExternalOutput")

    with TC(nc) as tc:
        with (
            tc.tile_pool(name="sb", bufs=1) as sb,
            tc.tile_pool(name="gp", bufs=1) as gp,
            tc.tile_pool(name="ps", bufs=1, space="PSUM") as ps,
            tc.tile_pool(name="dram", bufs=1, space="DRAM") as dram,
        ):
            _patch_pools(sb, gp, ps, dram)
            ident = sb.tile([128, 128], F32)
            make_identity(nc, ident)

            def load(ap, shape, tag):
                h = sb.tile(shape, F32, tag=tag)
                nc.sync.dma_start(h[:], ap)
                return h

            # l0 w1T packed host-side as [64, 2, 128]
            w01t = load(w01[:], [64, 2, 128], "w01")
            w1s_l0 = [w01t[:, 0, :], w01t[:, 1, :]]
            w02t = load(w02[:], [128, 128], "w02")
            gb0t = load(gb0[:], [128, 4], "gb0")
            w11t = [load(w11[i], [128, 256], f"w11_{i}") for i in range(2)]
            w12t = [load(w12[i], [128, 256], f"w12_{i}") for i in range(2)]
            gb1t = load(gb1_[:], [128, 8], "gb1_")
            pw1t = [load(pw1[i], [128, 256], f"pw1_{i}") for i in range(2)]
            pw2t = [load(pw2[i], [128, 256], f"pw2_{i}") for i in range(2)]
            gbpwt = load(gbpw[:], [128, 8], "gbpw")
            gbft = load(gbf[:], [128, 16], "gbf")
            sawqk = [[load(sa_wqk[li, cb], [128, 64], f"sawqk{li}_{cb}")
                      for cb in range(2)] for li in range(4)]
            sawv = [[load(sa_wv[li, cb], [128, 256], f"sawv{li}_{cb}")
                     for cb in range(2)] for li in range(4)]
            sawt = [[load(sa_wt[li, cb], [128, 256], f"sawt{li}_{cb}")
                     for cb in range(2)] for li in range(4)]
            sabtp = [[load(sa_btp[li, cb], [128, 1], f"sabtp{li}_{cb}")
                      for cb in range(2)] for li in range(4)]
            sagb = [load(sa_gb[li], [128, 4], f"sagb{li}") for li in range(4)]

            def transpose_sb(src_ap, tag):
                """PE-transpose src [P, Fw] -> sbuf [Fw, P]."""
                Pp = src_ap.shape[0]
                Fw = src_ap.shape[-1]
                pt = ps.tile([128, 128], F32, tag="Tps")
                nc.tensor.transpose(out=pt[:Fw, :Pp], in_=src_ap, identity=ident[:])
                st = sb.tile([128, 128], F32, tag=f"T_{tag}")
                nc.vector.tensor_copy(st[:Fw, :Pp], pt[:Fw, :Pp])
                return st

            sc = sb.tile([128, 1], F32, tag="g_sc")
            sqs = sb.tile([128, 256], F32, tag="g_sqs")

            def acc_stats(st, eb, zpsum):
                nc.scalar.activation(sqs[:, : zpsum.shape[-1]], zpsum, ACT.Copy,
                                     accum_out=sc[:])
                nc.vector.tensor_tensor(out=st[:, 2 * eb : 2 * eb + 1],
                                        in0=st[:, 2 * eb : 2 * eb + 1], in1=sc[:],
                                        op=OP.add)
                nc.scalar.activation(sqs[:, : zpsum.shape[-1]], zpsum, ACT.Square,
                                     accum_out=sc[:])
                nc.vector.tensor_tensor(out=st[:, 2 * eb + 1 : 2 * eb + 2],
                                        in0=st[:, 2 * eb + 1 : 2 * eb + 2], in1=sc[:],
                                        op=OP.add)

            # =============== grouped local op (l0 and l1) ===============
            def local_op(src_rows, ixg, ixc, CIN, COUT, S, K, w1s, w2s, gb, tag):
                NSB = S // 128
                NCB = (CIN + 127) // 128
                NEB = COUT // 128
                ixg_t, ixc_t = [], []
                for b in range(BL):
                    tg = sb.tile([128, NSB * K], I32, tag=f"{tag}_ixg{b}")
                    nc.sync.dma_start(tg[:], ixg[b, :, : NSB * K])
                    ixg_t.append(tg)
                    tcn = sb.tile([128, NSB], I32, tag=f"{tag}_ixc{b}")
                    nc.sync.dma_start(tcn[:], ixc[b, :, :NSB])
                    ixc_t.append(tcn)

                st1 = sb.tile([128, 2 * NEB], F32, tag=f"{tag}_st1")
                nc.vector.memset(st1[:], 0.0)
                st2 = sb.tile([128, 2 * NEB], F32, tag=f"{tag}_st2")
                nc.vector.memset(st2[:], 0.0)

                # centers: gather + transpose once, keep (small)
                cen = {}
                for b in range(BL):
                    for sblk in range(NSB):
                        cg = sb.tile([128, CIN], F32, tag=f"{tag}_cg{b}_{sblk}")
                        nc.gpsimd.indirect_dma_start(
                            out=cg[:], out_offset=None, in_=src_rows[:],
                            in_offset=bass.IndirectOffsetOnAxis(
                                ap=ixc_t[b][:, sblk : sblk + 1], axis=0))
                        ct = [transpose_sb(cg[:, cb * 128 : cb * 128 + min(128, CIN - cb * 128)],
                                           f"{tag}_cen{b}_{sblk}_{cb}")
                              for cb in range(NCB)]
                        cen[(b, sblk)] = (cg, ct)

                def gather(b, sblk):
                    g = gp.tile([128, K, CIN], F32, tag=f"{tag}_gath")
                    for k in range(K):
                        nc.gpsimd.indirect_dma_start(
                            out=g[:, k, :], out_offset=None, in_=src_rows[:],
                            in_offset=bass.IndirectOffsetOnAxis(
                                ap=ixg_t[b][:, sblk * K + k : sblk * K + k + 1],
                                axis=0))
                    return g

                def z1_psums(b, sblk, k, g):
                    cg, ct = cen[(b, sblk)]
                    sub = gp.tile([128, CIN], F32, tag=f"{tag}_sub")
                    nc.vector.tensor_tensor(out=sub[:], in0=g[:, k, :], in1=cg[:],
                                            op=OP.subtract)
                    uts = [transpose_sb(sub[:, cb * 128 : cb * 128 + min(128, CIN - cb * 128)],
                                        f"{tag}_ut{cb}") for cb in range(NCB)]
                    uts = uts + ct  # rows: (grouped-center | center)
                    zs = []
                    for eb in range(NEB):
                        z1 = ps.tile([128, 128], F32, tag=f"z1_{eb}")
                        nch = len(uts)
                        for ci, ut in enumerate(uts):
                            nc.tensor.matmul(
                                z1[:],
                                w1s[ci][:, eb * 128 : (eb + 1) * 128],
                                ut[: w1s[ci].shape[0], :],
                                start=(ci == 0), stop=(ci == nch - 1))
                        zs.append(z1)
                    return zs

                # ---- pass 1: bn1 stats ----
                for b in range(BL):
                    for sblk in range(NSB):
                        g = gather(b, sblk)
                        for k in range(K):
                            for eb, z1 in enumerate(z1_psums(b, sblk, k, g)):
                                acc_stats(st1, eb, z1[:])
                s1, t1 = _bn_coeffs(nc, sb, dram, st1, gb[:, : 2 * NEB],
                                    B * S * K, f"{tag}_bn1", G=NEB)

                # ---- pass 2: apply bn1+relu, mm2, running max, bn2 stats ----
                fmax = [[sb.tile([128, S], F32, tag=f"{tag}_f{b}_{eb}")
                         for eb in range(NEB)] for b in range(BL)]
                for b in range(BL):
                    for eb in range(NEB):
                        nc.vector.memset(fmax[b][eb][:], -1e30)
                for b in range(BL):
                    for sblk in range(NSB):
                        g = gather(b, sblk)
                        for k in range(K):
                            zs = z1_psums(b, sblk, k, g)
                            u2 = []
                            for eb, z1 in enumerate(zs):
                                u = gp.tile([128, 128], F32, tag=f"{tag}_u2_{eb}")
                                nc.scalar.activation(u[:], z1[:], ACT.Relu,
                                                     bias=t1[:, eb : eb + 1],
                                                     scale=s1[:, eb : eb + 1])
                                u2.append(u)
                            for eb in range(NEB):
                                y2 = ps.tile([128, 128], F32, tag=f"y2_{eb}")
                                for ci in range(NEB):
                                    nc.tensor.matmul(
                                        y2[:], w2s[ci][:, eb * 128 : (eb + 1) * 128],
                                        u2[ci][:], start=(ci == 0),
                                        stop=(ci == NEB - 1))
                                acc_stats(st2, eb, y2[:])
                                fs = fmax[b][eb][:, sblk * 128 : (sblk + 1) * 128]
                                nc.vector.tensor_tensor(out=fs, in0=fs, in1=y2[:],
                                                        op=OP.max)
                s2, t2 = _bn_coeffs(nc, sb, dram, st2, gb[:, 2 * NEB : 4 * NEB],
                                    B * S * K, f"{tag}_bn2", G=NEB)
                fout = [[sb.tile([128, S], F32, tag=f"{tag}_fo{b}_{eb}")
                         for eb in range(NEB)] for b in range(BL)]
                for b in range(BL):
                    for eb in range(NEB):
                        nc.scalar.activation(fout[b][eb][:], fmax[b][eb][:], ACT.Relu,
                                             bias=t2[:, eb : eb + 1],
                                             scale=s2[:, eb : eb + 1])
                return fout

            # l0: w1 chunks: rows 0:64 = grouped-center, 64:128 = center
            w2s_l0 = [w02t[:]]
            f0 = local_op(pts_rows, ix_g0, ix_c0, 64, 128, S1, K1,
                          w1s_l0, w2s_l0, gb0t[:], "l0")
            # f0[b][0] : [128, 512]

            # write f0 rows [BL*512, 128] to DRAM for l1 gathers
            f0_rows = dram.tile([BL * S1, 128], F32)
            for b in range(BL):
                for sblk in range(4):
                    tt = transpose_sb(f0[b][0][:, sblk * 128 : (sblk + 1) * 128],
                                      "f0r")
                    nc.sync.dma_start(
                        f0_rows[b * S1 + sblk * 128 : b * S1 + (sblk + 1) * 128, :],
                        tt[:])

            w1s_l1 = [w11t[0][:], w11t[1][:]]
            w2s_l1 = [w12t[0][:], w12t[1][:]]
            f1 = local_op(f0_rows, ix_g1, ix_c1, 128, 256, S2, K2,
                          w1s_l1, w2s_l1, gb1t[:], "l1")
            # f1[b][eb] : [128, 256], eb in {0,1}

            # =============== pointwise convs ===============
            def pointwise(xin, wts, gb, gcols, tag):
                """xin[b][cb] [128,256]; wts dram-loaded [2,128,256]; returns same shape."""
                st = sb.tile([128, 4], F32, tag=f"{tag}_st")
                nc.vector.memset(st[:], 0.0)
                zsb = [[sb.tile([128, 256], F32, tag=f"pwz_{b}_{eb}")
                        for eb in range(2)] for b in range(BL)]
                for b in range(BL):
                    for eb in range(2):
                        zp = ps.tile([128, 256], F32, tag="psA")
                        for ci in range(2):
                            nc.tensor.matmul(zp[:],
                                             wts[ci][:, eb * 128 : (eb + 1) * 128],
                                             xin[b][ci][:], start=(ci == 0),
                                             stop=(ci == 1))
                        nc.vector.tensor_copy(zsb[b][eb][:], zp[:])
                        acc_stats(st, eb, zp[:])
                s, t = _bn_coeffs(nc, sb, dram, st, gb[:, gcols : gcols + 4],
                                  B * 256, f"{tag}_bn", G=2)
                yout = [[sb.tile([128, 256], F32, tag=f"{tag}_y{b}_{eb}")
                         for eb in range(2)] for b in range(BL)]
                for b in range(BL):
                    for eb in range(2):
                        nc.scalar.activation(yout[b][eb][:], zsb[b][eb][:], ACT.Relu,
                                             bias=t[:, eb : eb + 1],
                                             scale=s[:, eb : eb + 1])
                return yout

            h = pointwise(f1, [pw1t[0][:], pw1t[1][:]], gbpwt, 0, "pw1")
            h = pointwise(h, [pw2t[0][:], pw2t[1][:]], gbpwt, 4, "pw2")

            # =============== 4 SA layers ===============
            # maintain x (c-major) and xT (n-major) per batch
            x_cur = h
            xT_cur = [[None, None] for _ in range(BL)]
            for b in range(BL):
                for nb in range(2):
                    xt_ = sb.tile([128, 256], F32, tag=f"sa_xt1_{b}_{nb}")
                    for cb in range(2):
                        tt = transpose_sb(x_cur[b][cb][:, nb * 128 : (nb + 1) * 128],
                                          "saT0")
                        nc.vector.tensor_copy(xt_[:, cb * 128 : (cb + 1) * 128],
                                              tt[:])
                    xT_cur[b][nb] = xt_
            sa_outputs = []

            for li in range(4):
                st = sb.tile([128, 4], F32, tag=f"sa{li}_st")
                nc.vector.memset(st[:], 0.0)
                z3_all = [[None, None] for _ in range(BL)]
                for b in range(BL):
                    x = x_cur[b]
                    xT = xT_cur[b]
                    # k = wqk @ x  [64, 256]
                    kp = ps.tile([64, 256], F32, tag="psA")
                    for cb in range(2):
                        nc.tensor.matmul(kp[:], sawqk[li][cb][:], x[cb][:],
                                         start=(cb == 0), stop=(cb == 1))
                    kq = sb.tile([64, 256], F32, tag="sa_kq")
                    nc.vector.tensor_copy(kq[:], kp[:])
                    ea_l, r_l = [], []
                    for nb in range(2):
                        ep = ps.tile([128, 256], F32, tag="psA")
                        nc.tensor.matmul(ep[:], kq[:, nb * 128 : (nb + 1) * 128],
                                         kq[:], start=True, stop=True)
                        rm = sb.tile([128, 1], F32, tag="sa_rm")
                        nc.vector.tensor_reduce(rm[:], ep[:], axis=AX.X, op=OP.max)
                        nc.vector.tensor_scalar_mul(rm[:], rm[:], -1.0)
                        ea = sb.tile([128, 256], F32, tag=f"sa_ea{nb}")
                        rs = sb.tile([128, 1], F32, tag=f"sa_rs{nb}")
                        nc.scalar.activation(ea[:], ep[:], ACT.Exp, bias=rm[:],
                                             accum_out=rs[:])
                        r = sb.tile([128, 1], F32, tag=f"sa_r{nb}")
                        nc.vector.reciprocal(r[:], rs[:])
                        ea_l.append(ea)
                        r_l.append(r)
                    # vT, scaled by row weight r
                    vTr_l = []
                    for nb in range(2):
                        vp = ps.tile([128, 256], F32, tag="psA")
                        for cb in range(2):
                            nc.tensor.matmul(
                                vp[:], x[cb][:, nb * 128 : (nb + 1) * 128],
                                sawv[li][cb][:], start=(cb == 0), stop=(cb == 1))
                        vTr = sb.tile([128, 256], F32, tag=f"sa_vTr{nb}")
                        nc.vector.tensor_scalar(vTr[:], vp[:], r_l[nb][:],
                                                scalar2=None, op0=OP.mult)
                        vTr_l.append(vTr)
                    # xrT (attention-weighted), column renorm, residual sub
                    resT_l = []
                    for mb in range(2):
                        xp = ps.tile([128, 256], F32, tag="psB")
                        for nb in range(2):
                            nc.tensor.matmul(
                                xp[:], ea_l[nb][:, mb * 128 : (mb + 1) * 128],
                                vTr_l[nb][:], start=(nb == 0), stop=(nb == 1))
                        csp = ps.tile([128, 1], F32, tag="psC")
                        for nb in range(2):
                            nc.tensor.matmul(
                                csp[:], ea_l[nb][:, mb * 128 : (mb + 1) * 128],
                                r_l[nb][:], start=(nb == 0), stop=(nb == 1))
                        cs = sb.tile([128, 1], F32, tag="sa_cs")
                        nc.vector.tensor_scalar_add(cs[:], csp[:], 1e-9)
                        scol = sb.tile([128, 1], F32, tag="sa_scol")
                        nc.vector.reciprocal(scol[:], cs[:])
                        tmp = sb.tile([128, 256], F32, tag="sa_tmp")
                        nc.vector.tensor_scalar(tmp[:], xp[:], scol[:],
                                                scalar2=None, op0=OP.mult)
                        resT = sb.tile([128, 256], F32, tag=f"sa_resT{mb}")
                        nc.vector.tensor_tensor(out=resT[:], in0=xT[mb][:],
                                                in1=tmp[:], op=OP.subtract)
                        resT_l.append(resT)
                    # res (c-major)
                    res_l = []
                    for cb in range(2):
                        rt = sb.tile([128, 256], F32, tag=f"sa_res{cb}")
                        for nb in range(2):
                            tt = transpose_sb(
                                resT_l[nb][:, cb * 128 : (cb + 1) * 128], "sa_rT")
                            nc.vector.tensor_copy(
                                rt[:, nb * 128 : (nb + 1) * 128], tt[:])
                        res_l.append(rt)
                    # xr2 = wt @ res + bt'
                    for eb in range(2):
                        zp = ps.tile([128, 256], F32, tag="psA")
                        for cb in range(2):
                            nc.tensor.matmul(
                                zp[:], sawt[li][cb][:, eb * 128 : (eb + 1) * 128],
                                res_l[cb][:], start=(cb == 0), stop=(cb == 1))
                        z3 = sb.tile([128, 256], F32, tag=f"sa_z3_{b}_{eb}")
                        # z3 = zp + bt'  (per-partition scalar add)
                        nc.vector.tensor_scalar(
                            z3[:], zp[:], sabtp[li][eb][:, 0:1], scalar2=None,
                            op0=OP.add)
                        acc_stats(st, eb, z3[:])
                        z3_all[b][eb] = z3
                s, t = _bn_coeffs(nc, sb, dram, st, sagb[li][:], B * 256,
                                  f"sa{li}_bn", G=2)
                x_new = [[None, None] for _ in range(BL)]
                xT_new = [[None, None] for _ in range(BL)]
                for b in range(BL):
                    for eb in range(2):
                        xr3 = sb.tile([128, 256], F32, tag="sa_xr3")
                        nc.scalar.activation(xr3[:], z3_all[b][eb][:], ACT.Relu,
                                             bias=t[:, eb : eb + 1],
                                             scale=s[:, eb : eb + 1])
                        xn = sb.tile([128, 256], F32, tag=f"sa{li}_xn{b}_{eb}")
                        nc.vector.tensor_tensor(out=xn[:], in0=x_cur[b][eb][:],
                                                in1=xr3[:], op=OP.add)
                        x_new[b][eb] = xn
                    for nb in range(2):
                        xt_ = sb.tile([128, 256], F32, tag=f"sa_xt{li % 2}_{b}_{nb}")
                        for cb in range(2):
                            tt = transpose_sb(
                                x_new[b][cb][:, nb * 128 : (nb + 1) * 128], "saTn")
                            nc.vector.tensor_copy(
                                xt_[:, cb * 128 : (cb + 1) * 128], tt[:])
                        xT_new[b][nb] = xt_
                x_cur = x_new
                xT_cur = xT_new
                sa_outputs.append(x_new)

            # =============== final head ===============
            # concat channel blocks: sa1(2), sa2(2), sa3(2), sa4(2), f1(2)
            stf = sb.tile([128, 16], F32, tag="f_st")
            nc.vector.memset(stf[:], 0.0)
            ymax = [sb.tile([128, 8], F32, tag=f"f_ymax{b}") for b in range(BL)]
            for b in range(BL):
                blocks = [sa_outputs[0][b][0], sa_outputs[0][b][1],
                          sa_outputs[1][b][0], sa_outputs[1][b][1],
                          sa_outputs[2][b][0], sa_outputs[2][b][1],
                          sa_outputs[3][b][0], sa_outputs[3][b][1],
                          f1[b][0], f1[b][1]]
                for eb in range(8):
                    yp = ps.tile([128, 256], F32, tag="psA")
                    for kb in range(10):
                        fwblk = gp.tile([128, 128], F32, tag="f_fwblk")
                        nc.sync.dma_start(fwblk[:], fw[kb, eb])
                        nc.tensor.matmul(yp[:], fwblk[:], blocks[kb][:],
                                         start=(kb == 0), stop=(kb == 9))
                    acc_stats(stf, eb, yp[:])
                    nc.vector.tensor_reduce(ymax[b][:, eb : eb + 1], yp[:],
                                            axis=AX.X, op=OP.max)
            sf, tf = _bn_coeffs(nc, sb, dram, stf, gbft[:], B * 256, "f_bn", G=8)
            for b in range(BL):
                ofin = sb.tile([128, 8], F32, tag="f_out")
                for eb in range(8):
                    nc.scalar.activation(ofin[:, eb : eb + 1],
                                         ymax[b][:, eb : eb + 1], ACT.Lrelu,
                                         bias=tf[:, eb : eb + 1],
                                         scale=sf[:, eb : eb + 1], alpha=0.2)
                for eb in range(8):
                    nc.sync.dma_start(out[b, eb], ofin[:, eb : eb + 1])
    nc.compile()
    _CACHE["l2"] = nc
    return nc


# ---------------------------------------------------------------------------
# host orchestration
# ---------------------------------------------------------------------------
def kernel(x, params):
    x = np.asarray(x, np.float32)
    p = {k: np.asarray(v, np.float32) if not isinstance(v, (dict, list)) else v
         for k, v in params.items()}

    def npa(v):
        return np.asarray(v, np.float32)

    # ---------- launch 1 ----------
    nc1 = build_l1()
    in_maps = []
    for c in range(N_CORES):
        xb = x[c * BL : (c + 1) * BL]                       # [BL, 2048, 7]
        x7 = xb.transpose(2, 0, 1).reshape(7, BL * N)       # [7, BL*N]
        in_maps.append({
            "x7": np.ascontiguousarray(x7),
            "w1T": np.ascontiguousarray(npa(p["w1"]).T),
            "w2T": np.ascontiguousarray(npa(p["w2"]).T),
            "gb1": np.stack([npa(p["g1"]), npa(p["b1"])], 1),
            "gb2": np.stack([npa(p["g2"]), npa(p["b2"])], 1),
        })
    res1 = run_bass_kernel_spmd(nc1, in_maps, list(range(N_CORES)))
    ptsT = np.concatenate([r["ptsT"] for r in res1.results], 0)  # [B, 64, N]
    pts = ptsT.transpose(0, 2, 1)                                # [B, N, 64]

    # ---------- host index selection ----------
    xyz = x[..., :3].astype(np.float32)
    fi1 = fps_np(xyz, S1)                                   # [B, 512]
    new_xyz1 = np.take_along_axis(xyz, fi1[..., None], 1)   # [B, 512, 3]
    knn1 = knn_np(new_xyz1, xyz, K1)                        # [B, 512, 32]
    fi2 = fps_np(new_xyz1, S2)                              # [B, 256]
    new_xyz2 = np.take_along_axis(new_xyz1, fi2[..., None], 1)
    knn2 = knn_np(new_xyz2, new_xyz1, K2)                   # [B, 256, 32]

    # ---------- launch 2 ----------
    nc2 = build_l2()

    def pack_gidx(knn, fi, S, K, NSB, n_src):
        """-> ix_g [BL,128,NSB*K], ix_c [BL,128,NSB] (global row ids)."""
        ix_g = np.zeros((BL, 128, NSB * K), np.int32)
        ix_c = np.zeros((BL, 128, NSB), np.int32)
        for b in range(BL):
            for sblk in range(NSB):
                s_ids = sblk * 128 + np.arange(128)
                ix_c[b, :, sblk] = fi[b, s_ids] + b * n_src
                for k in range(K):
                    ix_g[b, :, sblk * K + k] = knn[b, s_ids, k] + b * n_src
        return ix_g, ix_c

    sa = p["sa"]
    l0, l1p = p["l0"], p["l1"]

    def split_rows(m, nblk):
        return npa(m).reshape(nblk, 128, -1)

    def gb_blocks(g, bv, nblk):
        """[128, 2*nblk]: per block eb cols (g, b)."""
        g = npa(g).reshape(nblk, 128)
        bv = npa(bv).reshape(nblk, 128)
        out_ = np.zeros((128, 2 * nblk), np.float32)
        for eb in range(nblk):
            out_[:, 2 * eb] = g[eb]
            out_[:, 2 * eb + 1] = bv[eb]
        return out_

    in_maps2 = []
    for c in range(N_CORES):
        bs = slice(c * BL, (c + 1) * BL)
        g0, c0 = pack_gidx(knn1[bs], fi1[bs], S1, K1, 4, N)
        g1, c1 = pack_gidx(knn2[bs], fi2[bs], S2, K2, 2, S1)
        wqk = np.stack([split_rows(npa(s["wqk"]).T, 2) for s in sa])   # [4,2,128,64]
        wv = np.stack([split_rows(npa(s["wv"]).T, 2) for s in sa])
        wt = np.stack([split_rows(npa(s["wt"]).T, 2) for s in sa])
        btp = np.stack([
            split_rows((npa(s["bt"]) - npa(s["wt"]) @ npa(s["bv"]))[:, None], 2)
            for s in sa])                                              # [4,2,128,1]
        sagb_ = np.stack([gb_blocks(s["g"], s["b"], 2) for s in sa])   # [4,128,4]
        fwm = npa(p["fw"]).T.reshape(10, 128, 8, 128).transpose(0, 2, 1, 3)
        in_maps2.append({
            "pts_rows": np.ascontiguousarray(
                pts[bs].reshape(BL * N, 64)),
            "ix_g0": g0, "ix_c0": c0, "ix_g1": g1, "ix_c1": c1,
            "w01": np.ascontiguousarray(
                npa(l0["w1"]).T.reshape(2, 64, 128).transpose(1, 0, 2)),
            "w02": np.ascontiguousarray(npa(l0["w2"]).T),
            "gb0": np.concatenate(
                [gb_blocks(l0["g1"], l0["b1"], 1),
                 gb_blocks(l0["g2"], l0["b2"], 1)], 1),
            "w11": split_rows(npa(l1p["w1"]).T, 2),
            "w12": split_rows(npa(l1p["w2"]).T, 2),
            "gb1_": np.concatenate(
                [gb_blocks(l1p["g1"], l1p["b1"], 2),
                 gb_blocks(l1p["g2"], l1p["b2"], 2)], 1),
            "pw1": split_rows(npa(p["pw1"]).T, 2),
            "pw2": split_rows(npa(p["pw2"]).T, 2),
            "gbpw": np.concatenate(
                [gb_blocks(p["pg1"], p["pb1"], 2),
                 gb_blocks(p["pg2"], p["pb2"], 2)], 1),
            "sa_wqk": wqk, "sa_wv": wv, "sa_wt": wt, "sa_btp": btp,
            "sa_gb": sagb_,
            "fw": np.ascontiguousarray(fwm),
            "gbf": gb_blocks(p["fg"], p["fb"], 8),
        })
    res2 = run_bass_kernel_spmd(nc2, in_maps2, list(range(N_CORES)))
    outs = np.concatenate([r["out"] for r in res2.results], 0)  # [B, 8, 128]
    return outs.reshape(B, 1024)


# revision 22
# speedup vs baseline: 1.7421x; 1.7421x over previous
"""Trainium2 Bass kernel for nn_Branch_4887672782920 (PCT-style point cloud net).

Data-parallel over batch B=32 across 8 NeuronCores (4 clouds/core), two SPMD
launches with in-kernel cross-core AllReduce for the training-mode BatchNorm
statistics.  The serial farthest-point-sampling / kNN index selection (pure
fp32 index math) runs on host between the launches; all feature gathers,
matmuls, batchnorms, attention and pooling run on device.
"""
import numpy as np

import concourse.bass as bass
import concourse.bacc as bacc
import concourse.mybir as mybir
from concourse.bass_utils import run_bass_kernel_spmd
from concourse.masks import make_identity
from concourse.tile import TileContext
from concourse.vector_clock import ScopedClock

F32 = mybir.dt.float32
I32 = mybir.dt.int32
AX = mybir.AxisListType
OP = mybir.AluOpType
ACT = mybir.ActivationFunctionType

N_CORES = 8
B = 32
BL = B // N_CORES
N = 2048
S1, K1 = 512, 32
S2, K2 = 256, 32
EPS = 1e-5
RG = [list(range(N_CORES))]


class TC(TileContext):
    """Tail drain carries >2 sem waits; this walrus rejects that. Split it."""

    def _drain_and_barrier(self, tick_clock, wait_clock):
        import bass_rust

        drain_inst = self.nc.sync.drain()
        wait_clock.add_sem_waits(
            drain_inst.ins, ScopedClock({None: tick_clock.global_clock})
        )
        si = drain_inst.ins.sync_info
        waits = list(si.on_wait or [])
        if len(waits) > 2:
            si.on_wait = waits[:2]
            for i in range(2, len(waits), 2):
                d2 = self.nc.sync.drain()
                d2.ins.sync_info = bass_rust.SyncInfo(
                    on_wait=waits[i : i + 2], on_update=[]
                )
        self.nc.all_engine_barrier()
        assert self.sems is not None
        popped = self.nc._tile_sem_poison_stack.pop()
        assert popped is self._sem_poison
        self.nc.clear_and_free_semaphores(list(self.sems.allocated().values()))
        self.nc.all_engine_barrier()


def bc(ap, count, axis):
    """Stride-0 broadcast dim inserted at free position `axis` of an AP."""
    pairs = list(ap.ap)
    pairs.insert(axis, [0, count])
    return bass.AP(ap.tensor, ap.offset, pairs)


_CACHE = {}

_TILE_UID = [0]


def _patch_pools(*pools):
    for pool in pools:
        orig = pool.tile

        def tile(shape, dtype, _orig=orig, tag="", name=None, **kw):
            if name is None:
                if tag:
                    name = tag
                else:
                    _TILE_UID[0] += 1
                    name = f"u{_TILE_UID[0]}"
            return _orig(shape, dtype, tag=tag, name=name, **kw)

        pool.tile = tile


def _bn_coeffs(nc, sb, dram, stats, gb, count, tag, G=1):
    """AllReduce packed [128, 2G] (sum, sumsq per group) -> (s, t) [128, G]."""
    P = stats.shape[0]
    ib = dram.tile([P, 2 * G], F32, tag=f"{tag}_ib")
    ob = dram.tile([P, 2 * G], F32, tag=f"{tag}_ob")
    nc.sync.dma_start(ib[:], stats[:])
    nc.gpsimd.collective_compute(
        "AllReduce", OP.add, replica_groups=RG, ins=[ib[:].opt()], outs=[ob[:].opt()]
    )
    red = sb.tile([P, 2 * G], F32, tag=f"{tag}_red")
    nc.sync.dma_start(red[:], ob[:])
    s = sb.tile([P, G], F32, tag=f"{tag}_s")
    t = sb.tile([P, G], F32, tag=f"{tag}_t")
    mu = sb.tile([P, 1], F32, tag="bnc_mu")
    ex2 = sb.tile([P, 1], F32, tag="bnc_ex2")
    nvar = sb.tile([P, 1], F32, tag="bnc_nvar")
    sd = sb.tile([P, 1], F32, tag="bnc_sd")
    rstd = sb.tile([P, 1], F32, tag="bnc_rstd")
    tmp = sb.tile([P, 1], F32, tag="bnc_tmp")
    eps_t = sb.tile([P, 1], F32, tag="bnc_eps")
    nc.vector.memset(eps_t[:], EPS)
    for g in range(G):
        nc.vector.tensor_scalar_mul(mu[:], red[:, 2 * g : 2 * g + 1], 1.0 / count)
        nc.vector.tensor_scalar_mul(ex2[:], red[:, 2 * g + 1 : 2 * g + 2], 1.0 / count)
        nc.vector.scalar_tensor_tensor(
            out=nvar[:], in0=mu[:], scalar=mu[:], in1=ex2[:],
            op0=OP.mult, op1=OP.subtract)          # mu^2 - E[x^2]  (= -var)
        nc.scalar.activation(sd[:], nvar[:], ACT.Sqrt, bias=eps_t[:], scale=-1.0)
        nc.vector.reciprocal(rstd[:], sd[:])
        nc.vector.tensor_tensor(out=s[:, g : g + 1], in0=rstd[:],
                                in1=gb[:, 2 * g : 2 * g + 1], op=OP.mult)
        nc.vector.tensor_tensor(out=tmp[:], in0=mu[:], in1=s[:, g : g + 1], op=OP.mult)
        nc.vector.tensor_tensor(out=t[:, g : g + 1],
                                in0=gb[:, 2 * g + 1 : 2 * g + 2], in1=tmp[:],
                                op=OP.subtract)    # t = b - mu*s
    return s, t


# ---------------------------------------------------------------------------
# Launch 1: input MLP.  x7 [7, BL*2048] -> ptsT [BL, 64, 2048]
# ---------------------------------------------------------------------------
def build_l1():
    if "l1" in _CACHE:
        return _CACHE["l1"]
    nc = bacc.Bacc("TRN2", num_devices=N_CORES)
    x7 = nc.dram_tensor("x7", [7, BL * N], F32, kind="ExternalInput")
    w1T = nc.dram_tensor("w1T", [7, 64], F32, kind="ExternalInput")
    w2T = nc.dram_tensor("w2T", [64, 64], F32, kind="ExternalInput")
    gb1 = nc.dram_tensor("gb1", [64, 2], F32, kind="ExternalInput")
    gb2 = nc.dram_tensor("gb2", [64, 2], F32, kind="ExternalInput")
    ptsT_out = nc.dram_tensor("ptsT", [BL, 64, N], F32, kind="ExternalOutput")

    NCH = N // 512
    with TC(nc) as tc:
        with (
            tc.tile_pool(name="sb", bufs=1) as sb,
            tc.tile_pool(name="ps", bufs=4, space="PSUM") as ps,
            tc.tile_pool(name="dram", bufs=1, space="DRAM") as dram,
        ):
            _patch_pools(sb, ps, dram)
            xt = sb.tile([7, BL * N], F32)
            nc.sync.dma_start(xt[:], x7[:])
            w1t = sb.tile([7, 64], F32)
            nc.sync.dma_start(w1t[:], w1T[:])
            w2t = sb.tile([64, 64], F32)
            nc.sync.dma_start(w2t[:], w2T[:])
            gb1t = sb.tile([64, 2], F32)
            nc.sync.dma_start(gb1t[:], gb1[:])
            gb2t = sb.tile([64, 2], F32)
            nc.sync.dma_start(gb2t[:], gb2[:])
            sqs = sb.tile([64, 512], F32)
            sc = sb.tile([64, 1], F32)

            def layer(src, wt, gbt, tag):
                h = sb.tile([64, BL * N], F32, tag=f"{tag}_h")
                st = sb.tile([64, 2], F32, tag=f"{tag}_st")
                nc.vector.memset(st[:], 0.0)
                for b in range(BL):
                    for ch in range(NCH):
                        pt = ps.tile([64, 512], F32, tag="mm")
                        sl = slice((b * NCH + ch) * 512, (b * NCH + ch + 1) * 512)
                        nc.tensor.matmul(pt[:], wt[:], src[:, sl], start=True, stop=True)
                        nc.scalar.activation(h[:, sl], pt[:], ACT.Copy,
                                             accum_out=sc[:])
                        nc.vector.tensor_tensor(out=st[:, 0:1], in0=st[:, 0:1],
                                                in1=sc[:], op=OP.add)
                        nc.scalar.activation(sqs[:], pt[:], ACT.Square,
                                             accum_out=sc[:])
                        nc.vector.tensor_tensor(out=st[:, 1:2], in0=st[:, 1:2],
                                                in1=sc[:], op=OP.add)
                s, t = _bn_coeffs(nc, sb, dram, st, gbt[:], B * N, tag)
                hn = sb.tile([64, BL * N], F32, tag=f"{tag}_hn")
                for b in range(BL):
                    sl = slice(b * N, (b + 1) * N)
                    nc.scalar.activation(hn[:, sl], h[:, sl], ACT.Relu,
                                         bias=t[:, 0:1], scale=s[:, 0:1])
                return hn

            h1 = layer(xt, w1t, gb1t, "bn1")
            h2 = layer(h1, w2t, gb2t, "bn2")
            for b in range(BL):
                nc.sync.dma_start(ptsT_out[b], h2[:, b * N : (b + 1) * N])
    nc.compile()
    _CACHE["l1"] = nc
    return nc


# ---------------------------------------------------------------------------
# Host index math (FPS + kNN), replicating reference fp32 semantics.
# ---------------------------------------------------------------------------
def fps_np(xyz, npoint):
    Bn, Nn, _ = xyz.shape
    dist = np.full((Bn, Nn), 1e10, np.float32)
    far = np.zeros(Bn, np.int64)
    out = np.zeros((Bn, npoint), np.int64)
    ar = np.arange(Bn)
    diff = np.empty_like(xyz)
    d = np.empty((Bn, Nn), np.float32)
    for i in range(npoint):
        out[:, i] = far
        np.subtract(xyz, xyz[ar, far][:, None, :], out=diff)
        np.square(diff, out=diff)
        np.sum(diff, axis=-1, out=d)
        np.minimum(dist, d, out=dist)
        far = np.argmax(dist, axis=1)
    return out


def knn_np(new_xyz, xyz, k):
    d = (
        (new_xyz**2).sum(-1)[:, :, None]
        + (xyz**2).sum(-1)[:, None, :]
        - 2.0 * np.einsum("bsc,bnc->bsn", new_xyz, xyz)
    ).astype(np.float32)
    return np.argsort(d, axis=-1, kind="stable")[:, :, :k]


# ---------------------------------------------------------------------------
# Launch 2
# ---------------------------------------------------------------------------
def build_l2():
    if "l2" in _CACHE:
        return _CACHE["l2"]
    nc = bacc.Bacc("TRN2", num_devices=N_CORES)

    pts_rows = nc.dram_tensor("pts_rows", [BL * N, 64], F32, kind="ExternalInput")
    ix_g0 = nc.dram_tensor("ix_g0", [BL, 128, 4 * K1], I32, kind="ExternalInput")
    ix_c0 = nc.dram_tensor("ix_c0", [BL, 128, 4], I32, kind="ExternalInput")
    ix_g1 = nc.dram_tensor("ix_g1", [BL, 128, 2 * K2], I32, kind="ExternalInput")
    ix_c1 = nc.dram_tensor("ix_c1", [BL, 128, 2], I32, kind="ExternalInput")

    w01 = nc.dram_tensor("w01", [64, 2, 128], F32, kind="ExternalInput")
    w02 = nc.dram_tensor("w02", [128, 128], F32, kind="ExternalInput")
    gb0 = nc.dram_tensor("gb0", [128, 4], F32, kind="ExternalInput")
    w11 = nc.dram_tensor("w11", [2, 128, 256], F32, kind="ExternalInput")
    w12 = nc.dram_tensor("w12", [2, 128, 256], F32, kind="ExternalInput")
    gb1_ = nc.dram_tensor("gb1_", [128, 8], F32, kind="ExternalInput")
    pw1 = nc.dram_tensor("pw1", [2, 128, 256], F32, kind="ExternalInput")
    pw2 = nc.dram_tensor("pw2", [2, 128, 256], F32, kind="ExternalInput")
    gbpw = nc.dram_tensor("gbpw", [128, 8], F32, kind="ExternalInput")
    sa_wqk = nc.dram_tensor("sa_wqk", [4, 2, 128, 64], F32, kind="ExternalInput")
    sa_wv = nc.dram_tensor("sa_wv", [4, 2, 128, 256], F32, kind="ExternalInput")
    sa_wt = nc.dram_tensor("sa_wt", [4, 2, 128, 256], F32, kind="ExternalInput")
    sa_btp = nc.dram_tensor("sa_btp", [4, 2, 128, 1], F32, kind="ExternalInput")
    sa_gb = nc.dram_tensor("sa_gb", [4, 128, 4], F32, kind="ExternalInput")
    fw = nc.dram_tensor("fw", [10, 8, 128, 128], F32, kind="ExternalInput")
    gbf = nc.dram_tensor("gbf", [128, 16], F32, kind="ExternalInput")

    out = nc.dram_tensor("out", [BL, 8, 128], F32, kind="ExternalOutput")

    with TC(nc) as tc:
        with (
            tc.tile_pool(name="sb", bufs=1) as sb,
            tc.tile_pool(name="gp", bufs=1) as gp,
            tc.tile_pool(name="ps", bufs=1, space="PSUM") as ps,
            tc.tile_pool(name="dram", bufs=1, space="DRAM") as dram,
        ):
            _patch_pools(sb, gp, ps, dram)
            ident = sb.tile([128, 128], F32)
            make_identity(nc, ident)

            def load(ap, shape, tag):
                h = sb.tile(shape, F32, tag=tag)
                nc.sync.dma_start(h[:], ap)
                return h

            # l0 w1T packed host-side as [64, 2, 128]
            w01t = load(w01[:], [64, 2, 128], "w01")
            w1s_l0 = [w01t[:, 0, :], w01t[:, 1, :]]
            w02t = load(w02[:], [128, 128], "w02")
            gb0t = load(gb0[:], [128, 4], "gb0")
            w11t = [load(w11[i], [128, 256], f"w11_{i}") for i in range(2)]
            w12t = [load(w12[i], [128, 256], f"w12_{i}") for i in range(2)]
            gb1t = load(gb1_[:], [128, 8], "gb1_")
            pw1t = [load(pw1[i], [128, 256], f"pw1_{i}") for i in range(2)]
            pw2t = [load(pw2[i], [128, 256], f"pw2_{i}") for i in range(2)]
            gbpwt = load(gbpw[:], [128, 8], "gbpw")
            gbft = load(gbf[:], [128, 16], "gbf")
            sawqk = [[load(sa_wqk[li, cb], [128, 64], f"sawqk{li}_{cb}")
                      for cb in range(2)] for li in range(4)]
            sawv = [[load(sa_wv[li, cb], [128, 256], f"sawv{li}_{cb}")
                     for cb in range(2)] for li in range(4)]
            sawt = [[load(sa_wt[li, cb], [128, 256], f"sawt{li}_{cb}")
                     for cb in range(2)] for li in range(4)]
            sabtp = [[load(sa_btp[li, cb], [128, 1], f"sabtp{li}_{cb}")
                      for cb in range(2)] for li in range(4)]
            sagb = [load(sa_gb[li], [128, 4], f"sagb{li}") for li in range(4)]

            def transpose_sb(src_ap, tag):
                """PE-transpose src [P, Fw] -> sbuf [Fw, P]."""
                Pp = src_ap.shape[0]
                Fw = src_ap.shape[-1]
                pt = ps.tile([128, 128], F32, tag="Tps")
                nc.tensor.transpose(out=pt[:Fw, :Pp], in_=src_ap, identity=ident[:])
                st = sb.tile([128, 128], F32, tag=f"T_{tag}")
                nc.vector.tensor_copy(st[:Fw, :Pp], pt[:Fw, :Pp])
                return st

            sc = sb.tile([128, 1], F32, tag="g_sc")
            sqs = sb.tile([128, 256], F32, tag="g_sqs")

            def acc_stats(st, eb, zpsum):
                nc.scalar.activation(sqs[:, : zpsum.shape[-1]], zpsum, ACT.Copy,
                                     accum_out=sc[:])
                nc.vector.tensor_tensor(out=st[:, 2 * eb : 2 * eb + 1],
                                        in0=st[:, 2 * eb : 2 * eb + 1], in1=sc[:],
                                        op=OP.add)
                nc.scalar.activation(sqs[:, : zpsum.shape[-1]], zpsum, ACT.Square,
                                     accum_out=sc[:])
                nc.vector.tensor_tensor(out=st[:, 2 * eb + 1 : 2 * eb + 2],
                                        in0=st[:, 2 * eb + 1 : 2 * eb + 2], in1=sc[:],
                                        op=OP.add)

            # =============== grouped local op (l0 and l1) ===============
            def local_op(src_rows, ixg, ixc, CIN, COUT, S, K, w1s, w2s, gb, tag):
                NSB = S // 128
                NCB = (CIN + 127) // 128
                NEB = COUT // 128
                ixg_t, ixc_t = [], []
                for b in range(BL):
                    tg = sb.tile([128, NSB * K], I32, tag=f"{tag}_ixg{b}")
                    nc.sync.dma_start(tg[:], ixg[b, :, : NSB * K])
                    ixg_t.append(tg)
                    tcn = sb.tile([128, NSB], I32, tag=f"{tag}_ixc{b}")
                    nc.sync.dma_start(tcn[:], ixc[b, :, :NSB])
                    ixc_t.append(tcn)

                st1 = sb.tile([128, 2 * NEB], F32, tag=f"{tag}_st1")
                nc.vector.memset(st1[:], 0.0)
                st2 = sb.tile([128, 2 * NEB], F32, tag=f"{tag}_st2")
                nc.vector.memset(st2[:], 0.0)

                # centers: gather + transpose once, keep (small)
                cen = {}
                for b in range(BL):
                    for sblk in range(NSB):
                        cg = sb.tile([128, CIN], F32, tag=f"{tag}_cg{b}_{sblk}")
                        nc.gpsimd.indirect_dma_start(
                            out=cg[:], out_offset=None, in_=src_rows[:],
                            in_offset=bass.IndirectOffsetOnAxis(
                                ap=ixc_t[b][:, sblk : sblk + 1], axis=0))
                        ct = [transpose_sb(cg[:, cb * 128 : cb * 128 + min(128, CIN - cb * 128)],
                                           f"{tag}_cen{b}_{sblk}_{cb}")
                              for cb in range(NCB)]
                        cen[(b, sblk)] = (cg, ct)

                def gather(b, sblk):
                    g = gp.tile([128, K, CIN], F32, tag=f"{tag}_gath")
                    for k in range(K):
                        nc.gpsimd.indirect_dma_start(
                            out=g[:, k, :], out_offset=None, in_=src_rows[:],
                            in_offset=bass.IndirectOffsetOnAxis(
                                ap=ixg_t[b][:, sblk * K + k : sblk * K + k + 1],
                                axis=0))
                    return g

                def z1_psums(b, sblk, k, g):
                    cg, ct = cen[(b, sblk)]
                    sub = gp.tile([128, CIN], F32, tag=f"{tag}_sub")
                    nc.vector.tensor_tensor(out=sub[:], in0=g[:, k, :], in1=cg[:],
                                            op=OP.subtract)
                    uts = [transpose_sb(sub[:, cb * 128 : cb * 128 + min(128, CIN - cb * 128)],
                                        f"{tag}_ut{cb}") for cb in range(NCB)]
                    uts = uts + ct  # rows: (grouped-center | center)
                    zs = []
                    for eb in range(NEB):
                        z1 = ps.tile([128, 128], F32, tag=f"z1_{eb}")
                        nch = len(uts)
                        for ci, ut in enumerate(uts):
                            nc.tensor.matmul(
                                z1[:],
                                w1s[ci][:, eb * 128 : (eb + 1) * 128],
                                ut[: w1s[ci].shape[0], :],
                                start=(ci == 0), stop=(ci == nch - 1))
                        zs.append(z1)
                    return zs

                # ---- pass 1: bn1 stats ----
                for b in range(BL):
                    for sblk in range(NSB):
                        g = gather(b, sblk)
                        for k in range(K):
                            for eb, z1 in enumerate(z1_psums(b, sblk, k, g)):
                                acc_stats(st1, eb, z1[:])
                s1, t1 = _bn_coeffs(nc, sb, dram, st1, gb[:, : 2 * NEB],
                                    B * S * K, f"{tag}_bn1", G=NEB)

                # ---- pass 2: apply bn1+relu, mm2, running max, bn2 stats ----
                fmax = [[sb.tile([128, S], F32, tag=f"{tag}_f{b}_{eb}")
                         for eb in range(NEB)] for b in range(BL)]
                for b in range(BL):
                    for eb in range(NEB):
                        nc.vector.memset(fmax[b][eb][:], -1e30)
                for b in range(BL):
                    for sblk in range(NSB):
                        g = gather(b, sblk)
                        for k in range(K):
                            zs = z1_psums(b, sblk, k, g)
                            u2 = []
                            for eb, z1 in enumerate(zs):
                                u = gp.tile([128, 128], F32, tag=f"{tag}_u2_{eb}")
                                nc.scalar.activation(u[:], z1[:], ACT.Relu,
                                                     bias=t1[:, eb : eb + 1],
                                                     scale=s1[:, eb : eb + 1])
                                u2.append(u)
                            for eb in range(NEB):
                                y2 = ps.tile([128, 128], F32, tag=f"y2_{eb}")
                                for ci in range(NEB):
                                    nc.tensor.matmul(
                                        y2[:], w2s[ci][:, eb * 128 : (eb + 1) * 128],
                                        u2[ci][:], start=(ci == 0),
                                        stop=(ci == NEB - 1))
                                acc_stats(st2, eb, y2[:])
                                fs = fmax[b][eb][:, sblk * 128 : (sblk + 1) * 128]
                                nc.vector.tensor_tensor(out=fs, in0=fs, in1=y2[:],
                                                        op=OP.max)
                s2, t2 = _bn_coeffs(nc, sb, dram, st2, gb[:, 2 * NEB : 4 * NEB],
                                    B * S * K, f"{tag}_bn2", G=NEB)
                fout = [[sb.tile([128, S], F32, tag=f"{tag}_fo{b}_{eb}")
                         for eb in range(NEB)] for b in range(BL)]
                for b in range(BL):
                    for eb in range(NEB):
                        nc.scalar.activation(fout[b][eb][:], fmax[b][eb][:], ACT.Relu,
                                             bias=t2[:, eb : eb + 1],
                                             scale=s2[:, eb : eb + 1])
                return fout

            # l0: w1 chunks: rows 0:64 = grouped-center, 64:128 = center
            w2s_l0 = [w02t[:]]
            f0 = local_op(pts_rows, ix_g0, ix_c0, 64, 128, S1, K1,
                          w1s_l0, w2s_l0, gb0t[:], "l0")
            # f0[b][0] : [128, 512]

            # write f0 rows [BL*512, 128] to DRAM for l1 gathers
            f0_rows = dram.tile([BL * S1, 128], F32)
            for b in range(BL):
                for sblk in range(4):
                    tt = transpose_sb(f0[b][0][:, sblk * 128 : (sblk + 1) * 128],
                                      "f0r")
                    nc.sync.dma_start(
                        f0_rows[b * S1 + sblk * 128 : b * S1 + (sblk + 1) * 128, :],
                        tt[:])

            w1s_l1 = [w11t[0][:], w11t[1][:]]
            w2s_l1 = [w12t[0][:], w12t[1][:]]
            f1 = local_op(f0_rows, ix_g1, ix_c1, 128, 256, S2, K2,
                          w1s_l1, w2s_l1, gb1t[:], "l1")
            # f1[b][eb] : [128, 256], eb in {0,1}

            # =============== pointwise convs ===============
            def pointwise(xin, wts, gb, gcols, tag):
                """xin[b][cb] [128,256]; wts dram-loaded [2,128,256]; returns same shape."""
                st = sb.tile([128, 4], F32, tag=f"{tag}_st")
                nc.vector.memset(st[:], 0.0)
                zsb = [[sb.tile([128, 256], F32, tag=f"pwz_{b}_{eb}")
                        for eb in range(2)] for b in range(BL)]
                for b in range(BL):
                    for eb in range(2):
                        zp = ps.tile([128, 256], F32, tag="psA")
                        for ci in range(2):
                            nc.tensor.matmul(zp[:],
                                             wts[ci][:, eb * 128 : (eb + 1) * 128],
                                             xin[b][ci][:], start=(ci == 0),
                                             stop=(ci == 1))
                        nc.vector.tensor_copy(zsb[b][eb][:], zp[:])
                        acc_stats(st, eb, zp[:])
                s, t = _bn_coeffs(nc, sb, dram, st, gb[:, gcols : gcols + 4],
                                  B * 256, f"{tag}_bn", G=2)
                yout = [[sb.tile([128, 256], F32, tag=f"{tag}_y{b}_{eb}")
                         for eb in range(2)] for b in range(BL)]
                for b in range(BL):
                    for eb in range(2):
                        nc.scalar.activation(yout[b][eb][:], zsb[b][eb][:], ACT.Relu,
                                             bias=t[:, eb : eb + 1],
                                             scale=s[:, eb : eb + 1])
                return yout

            h = pointwise(f1, [pw1t[0][:], pw1t[1][:]], gbpwt, 0, "pw1")
            h = pointwise(h, [pw2t[0][:], pw2t[1][:]], gbpwt, 4, "pw2")

            # =============== 4 SA layers ===============
            # maintain x (c-major) and xT (n-major) per batch
            x_cur = h
            xT_cur = [[None, None] for _ in range(BL)]
            for b in range(BL):
                for nb in range(2):
                    xt_ = sb.tile([128, 256], F32, tag=f"sa_xt1_{b}_{nb}")
                    for cb in range(2):
                        tt = transpose_sb(x_cur[b][cb][:, nb * 128 : (nb + 1) * 128],
                                          "saT0")
                        nc.vector.tensor_copy(xt_[:, cb * 128 : (cb + 1) * 128],
                                              tt[:])
                    xT_cur[b][nb] = xt_
            sa_outputs = []

            for li in range(4):
                st = sb.tile([128, 4], F32, tag=f"sa{li}_st")
                nc.vector.memset(st[:], 0.0)
                z3_all = [[None, None] for _ in range(BL)]
                for b in range(BL):
                    x = x_cur[b]
                    xT = xT_cur[b]
                    # k = wqk @ x  [64, 256]
                    kp = ps.tile([64, 256], F32, tag="psA")
                    for cb in range(2):
                        nc.tensor.matmul(kp[:], sawqk[li][cb][:], x[cb][:],
                                         start=(cb == 0), stop=(cb == 1))
                    kq = sb.tile([64, 256], F32, tag="sa_kq")
                    nc.vector.tensor_copy(kq[:], kp[:])
                    ea_l, r_l = [], []
                    for nb in range(2):
                        ep = ps.tile([128, 256], F32, tag="psA")
                        nc.tensor.matmul(ep[:], kq[:, nb * 128 : (nb + 1) * 128],
                                         kq[:], start=True, stop=True)
                        rm = sb.tile([128, 1], F32, tag="sa_rm")
                        nc.vector.tensor_reduce(rm[:], ep[:], axis=AX.X, op=OP.max)
                        nc.vector.tensor_scalar_mul(rm[:], rm[:], -1.0)
                        ea = sb.tile([128, 256], F32, tag=f"sa_ea{nb}")
                        rs = sb.tile([128, 1], F32, tag=f"sa_rs{nb}")
                        nc.scalar.activation(ea[:], ep[:], ACT.Exp, bias=rm[:],
                                             accum_out=rs[:])
                        r = sb.tile([128, 1], F32, tag=f"sa_r{nb}")
                        nc.vector.reciprocal(r[:], rs[:])
                        ea_l.append(ea)
                        r_l.append(r)
                    # vT, scaled by row weight r
                    vTr_l = []
                    for nb in range(2):
                        vp = ps.tile([128, 256], F32, tag="psA")
                        for cb in range(2):
                            nc.tensor.matmul(
                                vp[:], x[cb][:, nb * 128 : (nb + 1) * 128],
                                sawv[li][cb][:], start=(cb == 0), stop=(cb == 1))
                        vTr = sb.tile([128, 256], F32, tag=f"sa_vTr{nb}")
                        nc.vector.tensor_scalar(vTr[:], vp[:], r_l[nb][:],
                                                scalar2=None, op0=OP.mult)
                        vTr_l.append(vTr)
                    # xrT (attention-weighted), column renorm, residual sub
                    resT_l = []
                    for mb in range(2):
                        xp = ps.tile([128, 256], F32, tag="psB")
                        for nb in range(2):
                            nc.tensor.matmul(
                                xp[:], ea_l[nb][:, mb * 128 : (mb + 1) * 128],
                                vTr_l[nb][:], start=(nb == 0), stop=(nb == 1))
                        csp = ps.tile([128, 1], F32, tag="psC")
                        for nb in range(2):
                            nc.tensor.matmul(
                                csp[:], ea_l[nb][:, mb * 128 : (mb + 1) * 128],
                                r_l[nb][:], start=(nb == 0), stop=(nb == 1))
                        cs = sb.tile([128, 1], F32, tag="sa_cs")
                        nc.vector.tensor_scalar_add(cs[:], csp[:], 1e-9)
                        scol = sb.tile([128, 1], F32, tag="sa_scol")
                        nc.vector.reciprocal(scol[:], cs[:])
                        tmp = sb.tile([128, 256], F32, tag="sa_tmp")
                        nc.vector.tensor_scalar(tmp[:], xp[:], scol[:],
                                                scalar2=None, op0=OP.mult)
                        resT = sb.tile([128, 256], F32, tag=f"sa_resT{mb}")
                        nc.vector.tensor_tensor(out=resT[:], in0=xT[mb][:],
                                                in1=tmp[:], op=OP.subtract)
                        resT_l.append(resT)
                    # res (c-major)
                    res_l = []
                    for cb in range(2):
                        rt = sb.tile([128, 256], F32, tag=f"sa_res{cb}")
                        for nb in range(2):
                            tt = transpose_sb(
                                resT_l[nb][:, cb * 128 : (cb + 1) * 128], "sa_rT")
                            nc.vector.tensor_copy(
                                rt[:, nb * 128 : (nb + 1) * 128], tt[:])
                        res_l.append(rt)
                    # xr2 = wt @ res + bt'
                    for eb in range(2):
                        zp = ps.tile([128, 256], F32, tag="psA")
                        for cb in range(2):
                            nc.tensor.matmul(
                                zp[:], sawt[li][cb][:, eb * 128 : (eb + 1) * 128],
                                res_l[cb][:], start=(cb == 0), stop=(cb == 1))
                        z3 = sb.tile([128, 256], F32, tag=f"sa_z3_{b}_{eb}")
                        # z3 = zp + bt'  (per-partition scalar add)
                        nc.vector.tensor_scalar(
                            z3[:], zp[:], sabtp[li][eb][:, 0:1], scalar2=None,
                            op0=OP.add)
                        acc_stats(st, eb, z3[:])
                        z3_all[b][eb] = z3
                s, t = _bn_coeffs(nc, sb, dram, st, sagb[li][:], B * 256,
                                  f"sa{li}_bn", G=2)
                x_new = [[None, None] for _ in range(BL)]
                xT_new = [[None, None] for _ in range(BL)]
                for b in range(BL):
                    for eb in range(2):
                        xr3 = sb.tile([128, 256], F32, tag="sa_xr3")
                        nc.scalar.activation(xr3[:], z3_all[b][eb][:], ACT.Relu,
                                             bias=t[:, eb : eb + 1],
                                             scale=s[:, eb : eb + 1])
                        xn = sb.tile([128, 256], F32, tag=f"sa{li}_xn{b}_{eb}")
                        nc.vector.tensor_tensor(out=xn[:], in0=x_cur[b][eb][:],
                                                in1=xr3[:], op=OP.add)
                        x_new[b][eb] = xn
                    for nb in range(2):
                        xt_ = sb.tile([128, 256], F32, tag=f"sa_xt{li % 2}_{b}_{nb}")
                        for cb in range(2):
                            tt = transpose_sb(
                                x_new[b][cb][:, nb * 128 : (nb + 1) * 128], "saTn")
                            nc.vector.tensor_copy(
                                xt_[:, cb * 128 : (cb + 1) * 128], tt[:])
                        xT_new[b][nb] = xt_
                x_cur = x_new
                xT_cur = xT_new
                sa_outputs.append(x_new)

            # =============== final head ===============
            # concat channel blocks: sa1(2), sa2(2), sa3(2), sa4(2), f1(2)
            stf = sb.tile([128, 16], F32, tag="f_st")
            nc.vector.memset(stf[:], 0.0)
            ymax = [sb.tile([128, 8], F32, tag=f"f_ymax{b}") for b in range(BL)]
            for b in range(BL):
                blocks = [sa_outputs[0][b][0], sa_outputs[0][b][1],
                          sa_outputs[1][b][0], sa_outputs[1][b][1],
                          sa_outputs[2][b][0], sa_outputs[2][b][1],
                          sa_outputs[3][b][0], sa_outputs[3][b][1],
                          f1[b][0], f1[b][1]]
                for eb in range(8):
                    yp = ps.tile([128, 256], F32, tag="psA")
                    for kb in range(10):
                        fwblk = gp.tile([128, 128], F32, tag="f_fwblk")
                        nc.sync.dma_start(fwblk[:], fw[kb, eb])
                        nc.tensor.matmul(yp[:], fwblk[:], blocks[kb][:],
                                         start=(kb == 0), stop=(kb == 9))
                    acc_stats(stf, eb, yp[:])
                    nc.vector.tensor_reduce(ymax[b][:, eb : eb + 1], yp[:],
                                            axis=AX.X, op=OP.max)
            sf, tf = _bn_coeffs(nc, sb, dram, stf, gbft[:], B * 256, "f_bn", G=8)
            for b in range(BL):
                ofin = sb.tile([128, 8], F32, tag="f_out")
                for eb in range(8):
                    nc.scalar.activation(ofin[:, eb : eb + 1],
                                         ymax[b][:, eb : eb + 1], ACT.Lrelu,
                                         bias=tf[:, eb : eb + 1],
                                         scale=sf[:, eb : eb + 1], alpha=0.2)
                for eb in range(8):
                    nc.sync.dma_start(out[b, eb], ofin[:, eb : eb + 1])
    nc.compile()
    _CACHE["l2"] = nc
    return nc


# ---------------------------------------------------------------------------
# host orchestration
# ---------------------------------------------------------------------------
LAUNCH_NS = []


def kernel(x, params):
    import time as _time
    LAUNCH_NS.clear()
    x = np.asarray(x, np.float32)
    p = {k: np.asarray(v, np.float32) if not isinstance(v, (dict, list)) else v
         for k, v in params.items()}

    def npa(v):
        return np.asarray(v, np.float32)

    # ---------- launch 1 ----------
    nc1 = build_l1()
    in_maps = []
    for c in range(N_CORES):
        xb = x[c * BL : (c + 1) * BL]                       # [BL, 2048, 7]
        x7 = xb.transpose(2, 0, 1).reshape(7, BL * N)       # [7, BL*N]
        in_maps.append({
            "x7": np.ascontiguousarray(x7),
            "w1T": np.ascontiguousarray(npa(p["w1"]).T),
            "w2T": np.ascontiguousarray(npa(p["w2"]).T),
            "gb1": np.stack([npa(p["g1"]), npa(p["b1"])], 1),
            "gb2": np.stack([npa(p["g2"]), npa(p["b2"])], 1),
        })
    _t0 = _time.perf_counter()
    res1 = run_bass_kernel_spmd(nc1, in_maps, list(range(N_CORES)))
    LAUNCH_NS.append((_time.perf_counter() - _t0) * 1e9)
    ptsT = np.concatenate([r["ptsT"] for r in res1.results], 0)  # [B, 64, N]
    pts = ptsT.transpose(0, 2, 1)                                # [B, N, 64]

    # ---------- host index selection ----------
    xyz = x[..., :3].astype(np.float32)
    fi1 = fps_np(xyz, S1)                                   # [B, 512]
    new_xyz1 = np.take_along_axis(xyz, fi1[..., None], 1)   # [B, 512, 3]
    knn1 = knn_np(new_xyz1, xyz, K1)                        # [B, 512, 32]
    fi2 = fps_np(new_xyz1, S2)                              # [B, 256]
    new_xyz2 = np.take_along_axis(new_xyz1, fi2[..., None], 1)
    knn2 = knn_np(new_xyz2, new_xyz1, K2)                   # [B, 256, 32]

    # ---------- launch 2 ----------
    nc2 = build_l2()

    def pack_gidx(knn, fi, S, K, NSB, n_src):
        """-> ix_g [BL,128,NSB*K], ix_c [BL,128,NSB] (global row ids)."""
        ix_g = np.zeros((BL, 128, NSB * K), np.int32)
        ix_c = np.zeros((BL, 128, NSB), np.int32)
        for b in range(BL):
            for sblk in range(NSB):
                s_ids = sblk * 128 + np.arange(128)
                ix_c[b, :, sblk] = fi[b, s_ids] + b * n_src
                for k in range(K):
                    ix_g[b, :, sblk * K + k] = knn[b, s_ids, k] + b * n_src
        return ix_g, ix_c

    sa = p["sa"]
    l0, l1p = p["l0"], p["l1"]

    def split_rows(m, nblk):
        return npa(m).reshape(nblk, 128, -1)

    def gb_blocks(g, bv, nblk):
        """[128, 2*nblk]: per block eb cols (g, b)."""
        g = npa(g).reshape(nblk, 128)
        bv = npa(bv).reshape(nblk, 128)
        out_ = np.zeros((128, 2 * nblk), np.float32)
        for eb in range(nblk):
            out_[:, 2 * eb] = g[eb]
            out_[:, 2 * eb + 1] = bv[eb]
        return out_

    in_maps2 = []
    for c in range(N_CORES):
        bs = slice(c * BL, (c + 1) * BL)
        g0, c0 = pack_gidx(knn1[bs], fi1[bs], S1, K1, 4, N)
        g1, c1 = pack_gidx(knn2[bs], fi2[bs], S2, K2, 2, S1)
        wqk = np.stack([split_rows(npa(s["wqk"]).T, 2) for s in sa])   # [4,2,128,64]
        wv = np.stack([split_rows(npa(s["wv"]).T, 2) for s in sa])
        wt = np.stack([split_rows(npa(s["wt"]).T, 2) for s in sa])
        btp = np.stack([
            split_rows((npa(s["bt"]) - npa(s["wt"]) @ npa(s["bv"]))[:, None], 2)
            for s in sa])                                              # [4,2,128,1]
        sagb_ = np.stack([gb_blocks(s["g"], s["b"], 2) for s in sa])   # [4,128,4]
        fwm = npa(p["fw"]).T.reshape(10, 128, 8, 128).transpose(0, 2, 1, 3)
        in_maps2.append({
            "pts_rows": np.ascontiguousarray(
                pts[bs].reshape(BL * N, 64)),
            "ix_g0": g0, "ix_c0": c0, "ix_g1": g1, "ix_c1": c1,
            "w01": np.ascontiguousarray(
                npa(l0["w1"]).T.reshape(2, 64, 128).transpose(1, 0, 2)),
            "w02": np.ascontiguousarray(npa(l0["w2"]).T),
            "gb0": np.concatenate(
                [gb_blocks(l0["g1"], l0["b1"], 1),
                 gb_blocks(l0["g2"], l0["b2"], 1)], 1),
            "w11": split_rows(npa(l1p["w1"]).T, 2),
            "w12": split_rows(npa(l1p["w2"]).T, 2),
            "gb1_": np.concatenate(
                [gb_blocks(l1p["g1"], l1p["b1"], 2),
                 gb_blocks(l1p["g2"], l1p["b2"], 2)], 1),
            "pw1": split_rows(npa(p["pw1"]).T, 2),
            "pw2": split_rows(npa(p["pw2"]).T, 2),
            "gbpw": np.concatenate(
                [gb_blocks(p["pg1"], p["pb1"], 2),
                 gb_blocks(p["pg2"], p["pb2"], 2)], 1),
            "sa_wqk": wqk, "sa_wv": wv, "sa_wt": wt, "sa_btp": btp,
            "sa_gb": sagb_,
            "fw": np.ascontiguousarray(fwm),
            "gbf": gb_blocks(p["fg"], p["fb"], 8),
        })
    _t0 = _time.perf_counter()
    res2 = run_bass_kernel_spmd(nc2, in_maps2, list(range(N_CORES)))
    LAUNCH_NS.append((_time.perf_counter() - _t0) * 1e9)
    outs = np.concatenate([r["out"] for r in res2.results], 0)  # [B, 8, 128]
    return outs.reshape(B, 1024)


# revision 23
# speedup vs baseline: 4.8007x; 2.7557x over previous
"""Trainium2 Bass kernel for nn_Branch_4887672782920 (PCT-style point cloud net).

Data-parallel over batch B=32 across 8 NeuronCores (4 clouds/core), two SPMD
launches with in-kernel cross-core AllReduce for the training-mode BatchNorm
statistics.  The serial farthest-point-sampling / kNN index selection (pure
fp32 index math) runs on host between the launches; all feature gathers,
matmuls, batchnorms, attention and pooling run on device.
"""
import numpy as np

import concourse.bass as bass
import concourse.bacc as bacc
import concourse.mybir as mybir
from concourse.bass_utils import run_bass_kernel_spmd
from concourse.masks import make_identity
from concourse.tile import TileContext
from concourse.vector_clock import ScopedClock

F32 = mybir.dt.float32
I32 = mybir.dt.int32
AX = mybir.AxisListType
OP = mybir.AluOpType
ACT = mybir.ActivationFunctionType

N_CORES = 8
B = 32
BL = B // N_CORES
N = 2048
S1, K1 = 512, 32
S2, K2 = 256, 32
EPS = 1e-5
RG = [list(range(N_CORES))]


class TC(TileContext):
    """Tail drain carries >2 sem waits; this walrus rejects that. Split it."""

    def _drain_and_barrier(self, tick_clock, wait_clock):
        import bass_rust

        drain_inst = self.nc.sync.drain()
        wait_clock.add_sem_waits(
            drain_inst.ins, ScopedClock({None: tick_clock.global_clock})
        )
        si = drain_inst.ins.sync_info
        waits = list(si.on_wait or [])
        if len(waits) > 2:
            si.on_wait = waits[:2]
            for i in range(2, len(waits), 2):
                d2 = self.nc.sync.drain()
                d2.ins.sync_info = bass_rust.SyncInfo(
                    on_wait=waits[i : i + 2], on_update=[]
                )
        self.nc.all_engine_barrier()
        assert self.sems is not None
        popped = self.nc._tile_sem_poison_stack.pop()
        assert popped is self._sem_poison
        self.nc.clear_and_free_semaphores(list(self.sems.allocated().values()))
        self.nc.all_engine_barrier()


def bc(ap, count, axis):
    """Stride-0 broadcast dim inserted at free position `axis` of an AP."""
    pairs = list(ap.ap)
    pairs.insert(axis, [0, count])
    return bass.AP(ap.tensor, ap.offset, pairs)


_CACHE = {}

_TILE_UID = [0]


def _patch_pools(*pools):
    for pool in pools:
        orig = pool.tile

        def tile(shape, dtype, _orig=orig, tag="", name=None, **kw):
            if name is None:
                if tag:
                    name = tag
                else:
                    _TILE_UID[0] += 1
                    name = f"u{_TILE_UID[0]}"
            return _orig(shape, dtype, tag=tag, name=name, **kw)

        pool.tile = tile


def _bn_coeffs(nc, sb, dram, stats, gb, count, tag, G=1):
    """AllReduce packed [128, 2G] (sum, sumsq per group) -> (s, t) [128, G]."""
    P = stats.shape[0]
    ib = dram.tile([P, 2 * G], F32, tag=f"{tag}_ib")
    ob = dram.tile([P, 2 * G], F32, tag=f"{tag}_ob")
    nc.sync.dma_start(ib[:], stats[:])
    nc.gpsimd.collective_compute(
        "AllReduce", OP.add, replica_groups=RG, ins=[ib[:].opt()], outs=[ob[:].opt()]
    )
    red = sb.tile([P, 2 * G], F32, tag=f"{tag}_red")
    nc.sync.dma_start(red[:], ob[:])
    s = sb.tile([P, G], F32, tag=f"{tag}_s")
    t = sb.tile([P, G], F32, tag=f"{tag}_t")
    mu = sb.tile([P, 1], F32, tag="bnc_mu")
    ex2 = sb.tile([P, 1], F32, tag="bnc_ex2")
    nvar = sb.tile([P, 1], F32, tag="bnc_nvar")
    sd = sb.tile([P, 1], F32, tag="bnc_sd")
    rstd = sb.tile([P, 1], F32, tag="bnc_rstd")
    tmp = sb.tile([P, 1], F32, tag="bnc_tmp")
    eps_t = sb.tile([P, 1], F32, tag="bnc_eps")
    nc.vector.memset(eps_t[:], EPS)
    for g in range(G):
        nc.vector.tensor_scalar_mul(mu[:], red[:, 2 * g : 2 * g + 1], 1.0 / count)
        nc.vector.tensor_scalar_mul(ex2[:], red[:, 2 * g + 1 : 2 * g + 2], 1.0 / count)
        nc.vector.scalar_tensor_tensor(
            out=nvar[:], in0=mu[:], scalar=mu[:], in1=ex2[:],
            op0=OP.mult, op1=OP.subtract)          # mu^2 - E[x^2]  (= -var)
        nc.scalar.activation(sd[:], nvar[:], ACT.Sqrt, bias=eps_t[:], scale=-1.0)
        nc.vector.reciprocal(rstd[:], sd[:])
        nc.vector.tensor_tensor(out=s[:, g : g + 1], in0=rstd[:],
                                in1=gb[:, 2 * g : 2 * g + 1], op=OP.mult)
        nc.vector.tensor_tensor(out=tmp[:], in0=mu[:], in1=s[:, g : g + 1], op=OP.mult)
        nc.vector.tensor_tensor(out=t[:, g : g + 1],
                                in0=gb[:, 2 * g + 1 : 2 * g + 2], in1=tmp[:],
                                op=OP.subtract)    # t = b - mu*s
    return s, t


# ---------------------------------------------------------------------------
# Launch 1: input MLP.  x7 [7, BL*2048] -> ptsT [BL, 64, 2048]
# ---------------------------------------------------------------------------
def build_l1():
    if "l1" in _CACHE:
        return _CACHE["l1"]
    nc = bacc.Bacc("TRN2", num_devices=N_CORES)
    x7 = nc.dram_tensor("x7", [7, BL * N], F32, kind="ExternalInput")
    w1T = nc.dram_tensor("w1T", [7, 64], F32, kind="ExternalInput")
    w2T = nc.dram_tensor("w2T", [64, 64], F32, kind="ExternalInput")
    gb1 = nc.dram_tensor("gb1", [64, 2], F32, kind="ExternalInput")
    gb2 = nc.dram_tensor("gb2", [64, 2], F32, kind="ExternalInput")
    ptsT_out = nc.dram_tensor("ptsT", [BL, 64, N], F32, kind="ExternalOutput")

    NCH = N // 512
    with TC(nc) as tc:
        with (
            tc.tile_pool(name="sb", bufs=1) as sb,
            tc.tile_pool(name="ps", bufs=4, space="PSUM") as ps,
            tc.tile_pool(name="dram", bufs=1, space="DRAM") as dram,
        ):
            _patch_pools(sb, ps, dram)
            xt = sb.tile([7, BL * N], F32)
            nc.sync.dma_start(xt[:], x7[:])
            w1t = sb.tile([7, 64], F32)
            nc.sync.dma_start(w1t[:], w1T[:])
            w2t = sb.tile([64, 64], F32)
            nc.sync.dma_start(w2t[:], w2T[:])
            gb1t = sb.tile([64, 2], F32)
            nc.sync.dma_start(gb1t[:], gb1[:])
            gb2t = sb.tile([64, 2], F32)
            nc.sync.dma_start(gb2t[:], gb2[:])
            sqs = sb.tile([64, 512], F32)
            sc = sb.tile([64, 1], F32)

            def layer(src, wt, gbt, tag):
                h = sb.tile([64, BL * N], F32, tag=f"{tag}_h")
                st = sb.tile([64, 2], F32, tag=f"{tag}_st")
                nc.vector.memset(st[:], 0.0)
                for b in range(BL):
                    for ch in range(NCH):
                        pt = ps.tile([64, 512], F32, tag="mm")
                        sl = slice((b * NCH + ch) * 512, (b * NCH + ch + 1) * 512)
                        nc.tensor.matmul(pt[:], wt[:], src[:, sl], start=True, stop=True)
                        nc.scalar.activation(h[:, sl], pt[:], ACT.Copy,
                                             accum_out=sc[:])
                        nc.vector.tensor_tensor(out=st[:, 0:1], in0=st[:, 0:1],
                                                in1=sc[:], op=OP.add)
                        nc.scalar.activation(sqs[:], pt[:], ACT.Square,
                                             accum_out=sc[:])
                        nc.vector.tensor_tensor(out=st[:, 1:2], in0=st[:, 1:2],
                                                in1=sc[:], op=OP.add)
                s, t = _bn_coeffs(nc, sb, dram, st, gbt[:], B * N, tag)
                hn = sb.tile([64, BL * N], F32, tag=f"{tag}_hn")
                for b in range(BL):
                    sl = slice(b * N, (b + 1) * N)
                    nc.scalar.activation(hn[:, sl], h[:, sl], ACT.Relu,
                                         bias=t[:, 0:1], scale=s[:, 0:1])
                return hn

            h1 = layer(xt, w1t, gb1t, "bn1")
            h2 = layer(h1, w2t, gb2t, "bn2")
            for b in range(BL):
                nc.sync.dma_start(ptsT_out[b], h2[:, b * N : (b + 1) * N])
    nc.compile()
    _CACHE["l1"] = nc
    return nc


# ---------------------------------------------------------------------------
# Host index math (FPS + kNN), replicating reference fp32 semantics.
# ---------------------------------------------------------------------------
def fps_np(xyz, npoint):
    Bn, Nn, _ = xyz.shape
    dist = np.full((Bn, Nn), 1e10, np.float32)
    far = np.zeros(Bn, np.int64)
    out = np.zeros((Bn, npoint), np.int64)
    ar = np.arange(Bn)
    diff = np.empty_like(xyz)
    d = np.empty((Bn, Nn), np.float32)
    for i in range(npoint):
        out[:, i] = far
        np.subtract(xyz, xyz[ar, far][:, None, :], out=diff)
        np.square(diff, out=diff)
        np.sum(diff, axis=-1, out=d)
        np.minimum(dist, d, out=dist)
        far = np.argmax(dist, axis=1)
    return out


def knn_np(new_xyz, xyz, k):
    d = (
        (new_xyz**2).sum(-1)[:, :, None]
        + (xyz**2).sum(-1)[:, None, :]
        - 2.0 * np.einsum("bsc,bnc->bsn", new_xyz, xyz)
    ).astype(np.float32)
    return np.argsort(d, axis=-1, kind="stable")[:, :, :k]


# ---------------------------------------------------------------------------
# Launch 2
# ---------------------------------------------------------------------------
def build_l2():
    if "l2" in _CACHE:
        return _CACHE["l2"]
    nc = bacc.Bacc("TRN2", num_devices=N_CORES)

    pts_rows = nc.dram_tensor("pts_rows", [BL * N, 64], F32, kind="ExternalInput")
    ix_g0 = nc.dram_tensor("ix_g0", [BL, 128, 4 * K1], I32, kind="ExternalInput")
    ix_c0 = nc.dram_tensor("ix_c0", [BL, 128, 4], I32, kind="ExternalInput")
    ix_g1 = nc.dram_tensor("ix_g1", [BL, 128, 2 * K2], I32, kind="ExternalInput")
    ix_c1 = nc.dram_tensor("ix_c1", [BL, 128, 2], I32, kind="ExternalInput")

    w01 = nc.dram_tensor("w01", [64, 2, 128], F32, kind="ExternalInput")
    w02 = nc.dram_tensor("w02", [128, 128], F32, kind="ExternalInput")
    gb0 = nc.dram_tensor("gb0", [128, 4], F32, kind="ExternalInput")
    w11 = nc.dram_tensor("w11", [2, 128, 256], F32, kind="ExternalInput")
    w12 = nc.dram_tensor("w12", [2, 128, 256], F32, kind="ExternalInput")
    gb1_ = nc.dram_tensor("gb1_", [128, 8], F32, kind="ExternalInput")
    pw1 = nc.dram_tensor("pw1", [2, 128, 256], F32, kind="ExternalInput")
    pw2 = nc.dram_tensor("pw2", [2, 128, 256], F32, kind="ExternalInput")
    gbpw = nc.dram_tensor("gbpw", [128, 8], F32, kind="ExternalInput")
    sa_wqk = nc.dram_tensor("sa_wqk", [4, 2, 128, 64], F32, kind="ExternalInput")
    sa_wv = nc.dram_tensor("sa_wv", [4, 2, 128, 256], F32, kind="ExternalInput")
    sa_wt = nc.dram_tensor("sa_wt", [4, 2, 128, 256], F32, kind="ExternalInput")
    sa_btp = nc.dram_tensor("sa_btp", [4, 2, 128, 1], F32, kind="ExternalInput")
    sa_gb = nc.dram_tensor("sa_gb", [4, 128, 4], F32, kind="ExternalInput")
    fw = nc.dram_tensor("fw", [10, 8, 128, 128], F32, kind="ExternalInput")
    gbf = nc.dram_tensor("gbf", [128, 16], F32, kind="ExternalInput")

    out = nc.dram_tensor("out", [BL, 8, 128], F32, kind="ExternalOutput")

    with TC(nc) as tc:
        with (
            tc.tile_pool(name="sb", bufs=1) as sb,
            tc.tile_pool(name="gp", bufs=1) as gp,
            tc.tile_pool(name="ps", bufs=1, space="PSUM") as ps,
            tc.tile_pool(name="dram", bufs=1, space="DRAM") as dram,
        ):
            _patch_pools(sb, gp, ps, dram)
            ident = sb.tile([128, 128], F32)
            make_identity(nc, ident)

            def load(ap, shape, tag):
                h = sb.tile(shape, F32, tag=tag)
                nc.sync.dma_start(h[:], ap)
                return h

            # l0 w1T packed host-side as [64, 2, 128]
            w01t = load(w01[:], [64, 2, 128], "w01")
            w1s_l0 = [w01t[:, 0, :], w01t[:, 1, :]]
            w02t = load(w02[:], [128, 128], "w02")
            gb0t = load(gb0[:], [128, 4], "gb0")
            w11t = [load(w11[i], [128, 256], f"w11_{i}") for i in range(2)]
            w12t = [load(w12[i], [128, 256], f"w12_{i}") for i in range(2)]
            gb1t = load(gb1_[:], [128, 8], "gb1_")
            pw1t = [load(pw1[i], [128, 256], f"pw1_{i}") for i in range(2)]
            pw2t = [load(pw2[i], [128, 256], f"pw2_{i}") for i in range(2)]
            gbpwt = load(gbpw[:], [128, 8], "gbpw")
            gbft = load(gbf[:], [128, 16], "gbf")
            sawqk = [[load(sa_wqk[li, cb], [128, 64], f"sawqk{li}_{cb}")
                      for cb in range(2)] for li in range(4)]
            sawv = [[load(sa_wv[li, cb], [128, 256], f"sawv{li}_{cb}")
                     for cb in range(2)] for li in range(4)]
            sawt = [[load(sa_wt[li, cb], [128, 256], f"sawt{li}_{cb}")
                     for cb in range(2)] for li in range(4)]
            sabtp = [[load(sa_btp[li, cb], [128, 1], f"sabtp{li}_{cb}")
                      for cb in range(2)] for li in range(4)]
            sagb = [load(sa_gb[li], [128, 4], f"sagb{li}") for li in range(4)]

            def transpose_sb(src_ap, tag):
                """PE-transpose src [P, Fw] -> sbuf [Fw, P]."""
                Pp = src_ap.shape[0]
                Fw = src_ap.shape[-1]
                pt = ps.tile([128, 128], F32, tag="Tps")
                nc.tensor.transpose(out=pt[:Fw, :Pp], in_=src_ap, identity=ident[:])
                st = sb.tile([128, 128], F32, tag=f"T_{tag}")
                nc.vector.tensor_copy(st[:Fw, :Pp], pt[:Fw, :Pp])
                return st

            sc = sb.tile([128, 1], F32, tag="g_sc")
            sqs = sb.tile([128, 256], F32, tag="g_sqs")

            def acc_stats(st, eb, zpsum):
                nc.scalar.activation(sqs[:, : zpsum.shape[-1]], zpsum, ACT.Copy,
                                     accum_out=sc[:])
                nc.vector.tensor_tensor(out=st[:, 2 * eb : 2 * eb + 1],
                                        in0=st[:, 2 * eb : 2 * eb + 1], in1=sc[:],
                                        op=OP.add)
                nc.scalar.activation(sqs[:, : zpsum.shape[-1]], zpsum, ACT.Square,
                                     accum_out=sc[:])
                nc.vector.tensor_tensor(out=st[:, 2 * eb + 1 : 2 * eb + 2],
                                        in0=st[:, 2 * eb + 1 : 2 * eb + 2], in1=sc[:],
                                        op=OP.add)

            # =============== grouped local op (l0 and l1) ===============
            def local_op(src_rows, ixg, ixc, CIN, COUT, S, K, w1s, w2s, gb, tag):
                NSB = S // 128
                NCB = (CIN + 127) // 128
                NEB = COUT // 128
                ixg_t, ixc_t = [], []
                for b in range(BL):
                    tg = sb.tile([128, NSB * K], I32, tag=f"{tag}_ixg{b}")
                    nc.sync.dma_start(tg[:], ixg[b, :, : NSB * K])
                    ixg_t.append(tg)
                    tcn = sb.tile([128, NSB], I32, tag=f"{tag}_ixc{b}")
                    nc.sync.dma_start(tcn[:], ixc[b, :, :NSB])
                    ixc_t.append(tcn)

                st1 = sb.tile([128, 2 * NEB], F32, tag=f"{tag}_st1")
                nc.vector.memset(st1[:], 0.0)
                st2 = sb.tile([128, 2 * NEB], F32, tag=f"{tag}_st2")
                nc.vector.memset(st2[:], 0.0)

                # centers: gather + transpose once, keep (small)
                cen = {}
                for b in range(BL):
                    for sblk in range(NSB):
                        cg = sb.tile([128, CIN], F32, tag=f"{tag}_cg{b}_{sblk}")
                        nc.gpsimd.indirect_dma_start(
                            out=cg[:], out_offset=None, in_=src_rows[:],
                            in_offset=bass.IndirectOffsetOnAxis(
                                ap=ixc_t[b][:, sblk : sblk + 1], axis=0))
                        ct = [transpose_sb(cg[:, cb * 128 : cb * 128 + min(128, CIN - cb * 128)],
                                           f"{tag}_cen{b}_{sblk}_{cb}")
                              for cb in range(NCB)]
                        cen[(b, sblk)] = (cg, ct)

                def gather(b, sblk):
                    g = gp.tile([128, K, CIN], F32, tag=f"{tag}_gath")
                    for k in range(K):
                        nc.gpsimd.indirect_dma_start(
                            out=g[:, k, :], out_offset=None, in_=src_rows[:],
                            in_offset=bass.IndirectOffsetOnAxis(
                                ap=ixg_t[b][:, sblk * K + k : sblk * K + k + 1],
                                axis=0))
                    return g

                def z1_psums(b, sblk, k, g):
                    cg, ct = cen[(b, sblk)]
                    sub = gp.tile([128, CIN], F32, tag=f"{tag}_sub")
                    nc.vector.tensor_tensor(out=sub[:], in0=g[:, k, :], in1=cg[:],
                                            op=OP.subtract)
                    uts = [transpose_sb(sub[:, cb * 128 : cb * 128 + min(128, CIN - cb * 128)],
                                        f"{tag}_ut{cb}") for cb in range(NCB)]
                    uts = uts + ct  # rows: (grouped-center | center)
                    zs = []
                    for eb in range(NEB):
                        z1 = ps.tile([128, 128], F32, tag=f"z1_{eb}")
                        nch = len(uts)
                        for ci, ut in enumerate(uts):
                            nc.tensor.matmul(
                                z1[:],
                                w1s[ci][:, eb * 128 : (eb + 1) * 128],
                                ut[: w1s[ci].shape[0], :],
                                start=(ci == 0), stop=(ci == nch - 1))
                        zs.append(z1)
                    return zs

                # ---- pass 1: bn1 stats ----
                for b in range(BL):
                    for sblk in range(NSB):
                        g = gather(b, sblk)
                        for k in range(K):
                            for eb, z1 in enumerate(z1_psums(b, sblk, k, g)):
                                acc_stats(st1, eb, z1[:])
                s1, t1 = _bn_coeffs(nc, sb, dram, st1, gb[:, : 2 * NEB],
                                    B * S * K, f"{tag}_bn1", G=NEB)

                # ---- pass 2: apply bn1+relu, mm2, running max, bn2 stats ----
                fmax = [[sb.tile([128, S], F32, tag=f"{tag}_f{b}_{eb}")
                         for eb in range(NEB)] for b in range(BL)]
                for b in range(BL):
                    for eb in range(NEB):
                        nc.vector.memset(fmax[b][eb][:], -1e30)
                for b in range(BL):
                    for sblk in range(NSB):
                        g = gather(b, sblk)
                        for k in range(K):
                            zs = z1_psums(b, sblk, k, g)
                            u2 = []
                            for eb, z1 in enumerate(zs):
                                u = gp.tile([128, 128], F32, tag=f"{tag}_u2_{eb}")
                                nc.scalar.activation(u[:], z1[:], ACT.Relu,
                                                     bias=t1[:, eb : eb + 1],
                                                     scale=s1[:, eb : eb + 1])
                                u2.append(u)
                            for eb in range(NEB):
                                y2 = ps.tile([128, 128], F32, tag=f"y2_{eb}")
                                for ci in range(NEB):
                                    nc.tensor.matmul(
                                        y2[:], w2s[ci][:, eb * 128 : (eb + 1) * 128],
                                        u2[ci][:], start=(ci == 0),
                                        stop=(ci == NEB - 1))
                                acc_stats(st2, eb, y2[:])
                                fs = fmax[b][eb][:, sblk * 128 : (sblk + 1) * 128]
                                nc.vector.tensor_tensor(out=fs, in0=fs, in1=y2[:],
                                                        op=OP.max)
                s2, t2 = _bn_coeffs(nc, sb, dram, st2, gb[:, 2 * NEB : 4 * NEB],
                                    B * S * K, f"{tag}_bn2", G=NEB)
                fout = [[sb.tile([128, S], F32, tag=f"{tag}_fo{b}_{eb}")
                         for eb in range(NEB)] for b in range(BL)]
                for b in range(BL):
                    for eb in range(NEB):
                        nc.scalar.activation(fout[b][eb][:], fmax[b][eb][:], ACT.Relu,
                                             bias=t2[:, eb : eb + 1],
                                             scale=s2[:, eb : eb + 1])
                return fout

            # l0: w1 chunks: rows 0:64 = grouped-center, 64:128 = center
            w2s_l0 = [w02t[:]]
            f0 = local_op(pts_rows, ix_g0, ix_c0, 64, 128, S1, K1,
                          w1s_l0, w2s_l0, gb0t[:], "l0")
            # f0[b][0] : [128, 512]

            # write f0 rows [BL*512, 128] to DRAM for l1 gathers
            f0_rows = dram.tile([BL * S1, 128], F32)
            for b in range(BL):
                for sblk in range(4):
                    tt = transpose_sb(f0[b][0][:, sblk * 128 : (sblk + 1) * 128],
                                      "f0r")
                    nc.sync.dma_start(
                        f0_rows[b * S1 + sblk * 128 : b * S1 + (sblk + 1) * 128, :],
                        tt[:])

            w1s_l1 = [w11t[0][:], w11t[1][:]]
            w2s_l1 = [w12t[0][:], w12t[1][:]]
            f1 = local_op(f0_rows, ix_g1, ix_c1, 128, 256, S2, K2,
                          w1s_l1, w2s_l1, gb1t[:], "l1")
            # f1[b][eb] : [128, 256], eb in {0,1}

            # =============== pointwise convs ===============
            def pointwise(xin, wts, gb, gcols, tag):
                """xin[b][cb] [128,256]; wts dram-loaded [2,128,256]; returns same shape."""
                st = sb.tile([128, 4], F32, tag=f"{tag}_st")
                nc.vector.memset(st[:], 0.0)
                zsb = [[sb.tile([128, 256], F32, tag=f"pwz_{b}_{eb}")
                        for eb in range(2)] for b in range(BL)]
                for b in range(BL):
                    for eb in range(2):
                        zp = ps.tile([128, 256], F32, tag="psA")
                        for ci in range(2):
                            nc.tensor.matmul(zp[:],
                                             wts[ci][:, eb * 128 : (eb + 1) * 128],
                                             xin[b][ci][:], start=(ci == 0),
                                             stop=(ci == 1))
                        nc.vector.tensor_copy(zsb[b][eb][:], zp[:])
                        acc_stats(st, eb, zp[:])
                s, t = _bn_coeffs(nc, sb, dram, st, gb[:, gcols : gcols + 4],
                                  B * 256, f"{tag}_bn", G=2)
                yout = [[sb.tile([128, 256], F32, tag=f"{tag}_y{b}_{eb}")
                         for eb in range(2)] for b in range(BL)]
                for b in range(BL):
                    for eb in range(2):
                        nc.scalar.activation(yout[b][eb][:], zsb[b][eb][:], ACT.Relu,
                                             bias=t[:, eb : eb + 1],
                                             scale=s[:, eb : eb + 1])
                return yout

            h = pointwise(f1, [pw1t[0][:], pw1t[1][:]], gbpwt, 0, "pw1")
            h = pointwise(h, [pw2t[0][:], pw2t[1][:]], gbpwt, 4, "pw2")

            # =============== 4 SA layers ===============
            # maintain x (c-major) and xT (n-major) per batch
            x_cur = h
            xT_cur = [[None, None] for _ in range(BL)]
            for b in range(BL):
                for nb in range(2):
                    xt_ = sb.tile([128, 256], F32, tag=f"sa_xt1_{b}_{nb}")
                    for cb in range(2):
                        tt = transpose_sb(x_cur[b][cb][:, nb * 128 : (nb + 1) * 128],
                                          "saT0")
                        nc.vector.tensor_copy(xt_[:, cb * 128 : (cb + 1) * 128],
                                              tt[:])
                    xT_cur[b][nb] = xt_
            sa_outputs = []

            for li in range(4):
                st = sb.tile([128, 4], F32, tag=f"sa{li}_st")
                nc.vector.memset(st[:], 0.0)
                z3_all = [[None, None] for _ in range(BL)]
                for b in range(BL):
                    x = x_cur[b]
                    xT = xT_cur[b]
                    # k = wqk @ x  [64, 256]
                    kp = ps.tile([64, 256], F32, tag="psA")
                    for cb in range(2):
                        nc.tensor.matmul(kp[:], sawqk[li][cb][:], x[cb][:],
                                         start=(cb == 0), stop=(cb == 1))
                    kq = sb.tile([64, 256], F32, tag="sa_kq")
                    nc.vector.tensor_copy(kq[:], kp[:])
                    ea_l, r_l = [], []
                    for nb in range(2):
                        ep = ps.tile([128, 256], F32, tag="psA")
                        nc.tensor.matmul(ep[:], kq[:, nb * 128 : (nb + 1) * 128],
                                         kq[:], start=True, stop=True)
                        rm = sb.tile([128, 1], F32, tag="sa_rm")
                        nc.vector.tensor_reduce(rm[:], ep[:], axis=AX.X, op=OP.max)
                        nc.vector.tensor_scalar_mul(rm[:], rm[:], -1.0)
                        ea = sb.tile([128, 256], F32, tag=f"sa_ea{nb}")
                        rs = sb.tile([128, 1], F32, tag=f"sa_rs{nb}")
                        nc.scalar.activation(ea[:], ep[:], ACT.Exp, bias=rm[:],
                                             accum_out=rs[:])
                        r = sb.tile([128, 1], F32, tag=f"sa_r{nb}")
                        nc.vector.reciprocal(r[:], rs[:])
                        ea_l.append(ea)
                        r_l.append(r)
                    # vT, scaled by row weight r
                    vTr_l = []
                    for nb in range(2):
                        vp = ps.tile([128, 256], F32, tag="psA")
                        for cb in range(2):
                            nc.tensor.matmul(
                                vp[:], x[cb][:, nb * 128 : (nb + 1) * 128],
                                sawv[li][cb][:], start=(cb == 0), stop=(cb == 1))
                        vTr = sb.tile([128, 256], F32, tag=f"sa_vTr{nb}")
                        nc.vector.tensor_scalar(vTr[:], vp[:], r_l[nb][:],
                                                scalar2=None, op0=OP.mult)
                        vTr_l.append(vTr)
                    # xrT (attention-weighted), column renorm, residual sub
                    resT_l = []
                    for mb in range(2):
                        xp = ps.tile([128, 256], F32, tag="psB")
                        for nb in range(2):
                            nc.tensor.matmul(
                                xp[:], ea_l[nb][:, mb * 128 : (mb + 1) * 128],
                                vTr_l[nb][:], start=(nb == 0), stop=(nb == 1))
                        csp = ps.tile([128, 1], F32, tag="psC")
                        for nb in range(2):
                            nc.tensor.matmul(
                                csp[:], ea_l[nb][:, mb * 128 : (mb + 1) * 128],
                                r_l[nb][:], start=(nb == 0), stop=(nb == 1))
                        cs = sb.tile([128, 1], F32, tag="sa_cs")
                        nc.vector.tensor_scalar_add(cs[:], csp[:], 1e-9)
                        scol = sb.tile([128, 1], F32, tag="sa_scol")
                        nc.vector.reciprocal(scol[:], cs[:])
                        tmp = sb.tile([128, 256], F32, tag="sa_tmp")
                        nc.vector.tensor_scalar(tmp[:], xp[:], scol[:],
                                                scalar2=None, op0=OP.mult)
                        resT = sb.tile([128, 256], F32, tag=f"sa_resT{mb}")
                        nc.vector.tensor_tensor(out=resT[:], in0=xT[mb][:],
                                                in1=tmp[:], op=OP.subtract)
                        resT_l.append(resT)
                    # res (c-major)
                    res_l = []
                    for cb in range(2):
                        rt = sb.tile([128, 256], F32, tag=f"sa_res{cb}")
                        for nb in range(2):
                            tt = transpose_sb(
                                resT_l[nb][:, cb * 128 : (cb + 1) * 128], "sa_rT")
                            nc.vector.tensor_copy(
                                rt[:, nb * 128 : (nb + 1) * 128], tt[:])
                        res_l.append(rt)
                    # xr2 = wt @ res + bt'
                    for eb in range(2):
                        zp = ps.tile([128, 256], F32, tag="psA")
                        for cb in range(2):
                            nc.tensor.matmul(
                                zp[:], sawt[li][cb][:, eb * 128 : (eb + 1) * 128],
                                res_l[cb][:], start=(cb == 0), stop=(cb == 1))
                        z3 = sb.tile([128, 256], F32, tag=f"sa_z3_{b}_{eb}")
                        # z3 = zp + bt'  (per-partition scalar add)
                        nc.vector.tensor_scalar(
                            z3[:], zp[:], sabtp[li][eb][:, 0:1], scalar2=None,
                            op0=OP.add)
                        acc_stats(st, eb, z3[:])
                        z3_all[b][eb] = z3
                s, t = _bn_coeffs(nc, sb, dram, st, sagb[li][:], B * 256,
                                  f"sa{li}_bn", G=2)
                x_new = [[None, None] for _ in range(BL)]
                xT_new = [[None, None] for _ in range(BL)]
                for b in range(BL):
                    for eb in range(2):
                        xr3 = sb.tile([128, 256], F32, tag="sa_xr3")
                        nc.scalar.activation(xr3[:], z3_all[b][eb][:], ACT.Relu,
                                             bias=t[:, eb : eb + 1],
                                             scale=s[:, eb : eb + 1])
                        xn = sb.tile([128, 256], F32, tag=f"sa{li}_xn{b}_{eb}")
                        nc.vector.tensor_tensor(out=xn[:], in0=x_cur[b][eb][:],
                                                in1=xr3[:], op=OP.add)
                        x_new[b][eb] = xn
                    for nb in range(2):
                        xt_ = sb.tile([128, 256], F32, tag=f"sa_xt{li % 2}_{b}_{nb}")
                        for cb in range(2):
                            tt = transpose_sb(
                                x_new[b][cb][:, nb * 128 : (nb + 1) * 128], "saTn")
                            nc.vector.tensor_copy(
                                xt_[:, cb * 128 : (cb + 1) * 128], tt[:])
                        xT_new[b][nb] = xt_
                x_cur = x_new
                xT_cur = xT_new
                sa_outputs.append(x_new)

            # =============== final head ===============
            # concat channel blocks: sa1(2), sa2(2), sa3(2), sa4(2), f1(2)
            stf = sb.tile([128, 16], F32, tag="f_st")
            nc.vector.memset(stf[:], 0.0)
            ymax = [sb.tile([128, 8], F32, tag=f"f_ymax{b}") for b in range(BL)]
            for b in range(BL):
                blocks = [sa_outputs[0][b][0], sa_outputs[0][b][1],
                          sa_outputs[1][b][0], sa_outputs[1][b][1],
                          sa_outputs[2][b][0], sa_outputs[2][b][1],
                          sa_outputs[3][b][0], sa_outputs[3][b][1],
                          f1[b][0], f1[b][1]]
                for eb in range(8):
                    yp = ps.tile([128, 256], F32, tag="psA")
                    for kb in range(10):
                        fwblk = gp.tile([128, 128], F32, tag="f_fwblk")
                        nc.sync.dma_start(fwblk[:], fw[kb, eb])
                        nc.tensor.matmul(yp[:], fwblk[:], blocks[kb][:],
                                         start=(kb == 0), stop=(kb == 9))
                    acc_stats(stf, eb, yp[:])
                    nc.vector.tensor_reduce(ymax[b][:, eb : eb + 1], yp[:],
                                            axis=AX.X, op=OP.max)
            sf, tf = _bn_coeffs(nc, sb, dram, stf, gbft[:], B * 256, "f_bn", G=8)
            for b in range(BL):
                ofin = sb.tile([128, 8], F32, tag="f_out")
                for eb in range(8):
                    nc.scalar.activation(ofin[:, eb : eb + 1],
                                         ymax[b][:, eb : eb + 1], ACT.Lrelu,
                                         bias=tf[:, eb : eb + 1],
                                         scale=sf[:, eb : eb + 1], alpha=0.2)
                for eb in range(8):
                    nc.sync.dma_start(out[b, eb], ofin[:, eb : eb + 1])
    nc.compile()
    _CACHE["l2"] = nc
    return nc


# ---------------------------------------------------------------------------
# host orchestration
# ---------------------------------------------------------------------------

# ---------------------------------------------------------------------------
# cached SPMD runner: jit/trace once per Bass module, rerun cheaply
# ---------------------------------------------------------------------------
_RUN_CACHE = {}


def _run_spmd(nc, in_maps):
    key = id(nc)
    if key not in _RUN_CACHE:
        import jax
        from jax.sharding import Mesh, PartitionSpec
        from jax.experimental.shard_map import shard_map
        from concourse import bass2jax

        bass2jax.install_neuronx_cc_hook()
        partition_name = (
            nc.partition_id_tensor.name if nc.partition_id_tensor else None
        )
        in_names, out_names, out_avals, zero_outs = [], [], [], []
        for alloc in nc.m.functions[0].allocations:
            if not isinstance(alloc, mybir.MemoryLocationSet):
                continue
            name = alloc.memorylocations[0].name
            if alloc.kind == "ExternalInput":
                if name != partition_name:
                    in_names.append(name)
            elif alloc.kind == "ExternalOutput":
                shape = tuple(alloc.tensor_shape)
                dtype = mybir.dt.np(alloc.dtype)
                out_names.append(name)
                out_avals.append(jax.core.ShapedArray(shape, dtype))
                zero_outs.append(np.zeros(shape, dtype))
        n_params = len(in_names)
        n_outs = len(out_avals)
        all_in_names = list(in_names) + list(out_names)
        if partition_name is not None:
            all_in_names.append(partition_name)
        donate = tuple(range(n_params, n_params + n_outs))

        def _body(*args):
            operands = list(args)
            if partition_name is not None:
                operands.append(bass2jax.partition_id_tensor())
            outs = bass2jax._bass_exec_p.bind(
                *operands,
                out_avals=tuple(out_avals),
                in_names=tuple(all_in_names),
                out_names=tuple(out_names),
                lowering_input_output_aliases=(),
                sim_require_finite=True,
                sim_require_nnan=True,
                nc=nc,
            )
            return tuple(outs)

        devices = jax.devices()[:N_CORES]
        mesh = Mesh(np.asarray(devices), ("core",))
        in_specs = (PartitionSpec("core"),) * (n_params + n_outs)
        out_specs = (PartitionSpec("core"),) * n_outs
        sharded = jax.jit(
            shard_map(_body, mesh=mesh, in_specs=in_specs, out_specs=out_specs,
                      check_rep=False),
            donate_argnums=donate, keep_unused=True,
        )
        _RUN_CACHE[key] = (sharded, in_names, out_names, out_avals, zero_outs)
    sharded, in_names, out_names, out_avals, zero_outs = _RUN_CACHE[key]
    concat_in = [
        np.concatenate([np.asarray(in_maps[c][nm]) for c in range(N_CORES)], axis=0)
        for nm in in_names
    ]
    concat_zeros = [
        np.zeros((N_CORES * z.shape[0], *z.shape[1:]), z.dtype) for z in zero_outs
    ]
    out_arrs = sharded(*concat_in, *concat_zeros)
    return [
        {nm: np.asarray(out_arrs[i]).reshape(N_CORES, *out_avals[i].shape)[c]
         for i, nm in enumerate(out_names)}
        for c in range(N_CORES)
    ]


LAUNCH_NS = []


def kernel(x, params):
    import time as _time
    LAUNCH_NS.clear()
    x = np.asarray(x, np.float32)
    p = {k: np.asarray(v, np.float32) if not isinstance(v, (dict, list)) else v
         for k, v in params.items()}

    def npa(v):
        return np.asarray(v, np.float32)

    # ---------- launch 1 ----------
    nc1 = build_l1()
    in_maps = []
    for c in range(N_CORES):
        xb = x[c * BL : (c + 1) * BL]                       # [BL, 2048, 7]
        x7 = xb.transpose(2, 0, 1).reshape(7, BL * N)       # [7, BL*N]
        in_maps.append({
            "x7": np.ascontiguousarray(x7),
            "w1T": np.ascontiguousarray(npa(p["w1"]).T),
            "w2T": np.ascontiguousarray(npa(p["w2"]).T),
            "gb1": np.stack([npa(p["g1"]), npa(p["b1"])], 1),
            "gb2": np.stack([npa(p["g2"]), npa(p["b2"])], 1),
        })
    _t0 = _time.perf_counter()
    res1_list = _run_spmd(nc1, in_maps)
    LAUNCH_NS.append((_time.perf_counter() - _t0) * 1e9)
    ptsT = np.concatenate([r["ptsT"] for r in res1_list], 0)  # [B, 64, N]
    pts = ptsT.transpose(0, 2, 1)                                # [B, N, 64]

    # ---------- host index selection ----------
    xyz = x[..., :3].astype(np.float32)
    fi1 = fps_np(xyz, S1)                                   # [B, 512]
    new_xyz1 = np.take_along_axis(xyz, fi1[..., None], 1)   # [B, 512, 3]
    knn1 = knn_np(new_xyz1, xyz, K1)                        # [B, 512, 32]
    fi2 = fps_np(new_xyz1, S2)                              # [B, 256]
    new_xyz2 = np.take_along_axis(new_xyz1, fi2[..., None], 1)
    knn2 = knn_np(new_xyz2, new_xyz1, K2)                   # [B, 256, 32]

    # ---------- launch 2 ----------
    nc2 = build_l2()

    def pack_gidx(knn, fi, S, K, NSB, n_src):
        """-> ix_g [BL,128,NSB*K], ix_c [BL,128,NSB] (global row ids)."""
        ix_g = np.zeros((BL, 128, NSB * K), np.int32)
        ix_c = np.zeros((BL, 128, NSB), np.int32)
        for b in range(BL):
            for sblk in range(NSB):
                s_ids = sblk * 128 + np.arange(128)
                ix_c[b, :, sblk] = fi[b, s_ids] + b * n_src
                for k in range(K):
                    ix_g[b, :, sblk * K + k] = knn[b, s_ids, k] + b * n_src
        return ix_g, ix_c

    sa = p["sa"]
    l0, l1p = p["l0"], p["l1"]

    def split_rows(m, nblk):
        return npa(m).reshape(nblk, 128, -1)

    def gb_blocks(g, bv, nblk):
        """[128, 2*nblk]: per block eb cols (g, b)."""
        g = npa(g).reshape(nblk, 128)
        bv = npa(bv).reshape(nblk, 128)
        out_ = np.zeros((128, 2 * nblk), np.float32)
        for eb in range(nblk):
            out_[:, 2 * eb] = g[eb]
            out_[:, 2 * eb + 1] = bv[eb]
        return out_

    in_maps2 = []
    for c in range(N_CORES):
        bs = slice(c * BL, (c + 1) * BL)
        g0, c0 = pack_gidx(knn1[bs], fi1[bs], S1, K1, 4, N)
        g1, c1 = pack_gidx(knn2[bs], fi2[bs], S2, K2, 2, S1)
        wqk = np.stack([split_rows(npa(s["wqk"]).T, 2) for s in sa])   # [4,2,128,64]
        wv = np.stack([split_rows(npa(s["wv"]).T, 2) for s in sa])
        wt = np.stack([split_rows(npa(s["wt"]).T, 2) for s in sa])
        btp = np.stack([
            split_rows((npa(s["bt"]) - npa(s["wt"]) @ npa(s["bv"]))[:, None], 2)
            for s in sa])                                              # [4,2,128,1]
        sagb_ = np.stack([gb_blocks(s["g"], s["b"], 2) for s in sa])   # [4,128,4]
        fwm = npa(p["fw"]).T.reshape(10, 128, 8, 128).transpose(0, 2, 1, 3)
        in_maps2.append({
            "pts_rows": np.ascontiguousarray(
                pts[bs].reshape(BL * N, 64)),
            "ix_g0": g0, "ix_c0": c0, "ix_g1": g1, "ix_c1": c1,
            "w01": np.ascontiguousarray(
                npa(l0["w1"]).T.reshape(2, 64, 128).transpose(1, 0, 2)),
            "w02": np.ascontiguousarray(npa(l0["w2"]).T),
            "gb0": np.concatenate(
                [gb_blocks(l0["g1"], l0["b1"], 1),
                 gb_blocks(l0["g2"], l0["b2"], 1)], 1),
            "w11": split_rows(npa(l1p["w1"]).T, 2),
            "w12": split_rows(npa(l1p["w2"]).T, 2),
            "gb1_": np.concatenate(
                [gb_blocks(l1p["g1"], l1p["b1"], 2),
                 gb_blocks(l1p["g2"], l1p["b2"], 2)], 1),
            "pw1": split_rows(npa(p["pw1"]).T, 2),
            "pw2": split_rows(npa(p["pw2"]).T, 2),
            "gbpw": np.concatenate(
                [gb_blocks(p["pg1"], p["pb1"], 2),
                 gb_blocks(p["pg2"], p["pb2"], 2)], 1),
            "sa_wqk": wqk, "sa_wv": wv, "sa_wt": wt, "sa_btp": btp,
            "sa_gb": sagb_,
            "fw": np.ascontiguousarray(fwm),
            "gbf": gb_blocks(p["fg"], p["fb"], 8),
        })
    _t0 = _time.perf_counter()
    res2_list = _run_spmd(nc2, in_maps2)
    LAUNCH_NS.append((_time.perf_counter() - _t0) * 1e9)
    outs = np.concatenate([r["out"] for r in res2_list], 0)  # [B, 8, 128]
    return outs.reshape(B, 1024)
